# revision 5
# baseline (speedup 1.0000x reference)
"""Trainium2 Bass kernel for nn_Nonhier_Optmatch (8-core SPMD).

Contract: kernel(**inputs) takes the FULL unsharded inputs and returns the
FULL [8192, 1] float32 output. Internally shards the N=262144 row axis
across 8 NeuronCores.

Math notes (exact reformulations of the reference):
  - The MHA softmax is over a size-1 axis => attn weight == 1.0, so q/k/Wq/Wk
    are dead code and attn_out = (x @ Wv.T + bv) @ Wo.T + bo.
  - Linear folding: member_emb = relu(x @ Wbig.T + bbig) with
    Wbig = Wfc @ Wo @ Wv and bbig = Wfc @ (Wo @ bv + bo) + bfc.
  - Output fold: out[s] = (sum_{i in s} relu(z_i) . wout) / count_s + bout,
    so only the scalar y_i = relu(z_i) . wout leaves the device; the ragged
    per-season mean over scalars is a host bincount.
  - With D1 = S - H, D2 = R - H and u = w1H + w1S:
      e1 = u.H + w1S.D1, e2 = u.H + w1S.D2, e3 = u.H
      agg = H + a1*D1 + a2*D2   (a = softmax(leaky_relu(e)))
"""
import sys

sys.path.insert(0, '/opt/trn_rl_repo')

import numpy as np

N_CORES = 8
N_TOTAL = 262144
ROWS = N_TOTAL // N_CORES        # 32768 rows per core
RBLK = 512                       # rows per block (PSUM bank width in fp32)
CF = 256
EMB = 256
NUM_SEASONS = 8192

_cache = {}


def _build_nc(rows):
    import concourse.bacc as bacc
    import concourse.tile as tile
    import concourse.mybir as mybir

    dt = mybir.dt
    op = mybir.AluOpType
    act = mybir.ActivationFunctionType
    nblk = rows // RBLK

    nc = bacc.Bacc("TRN2", target_bir_lowering=False, debug=False,
                   num_devices=N_CORES)

    xin_d = nc.dram_tensor("xin", [1024, rows], dt.float32r, kind="ExternalInput")
    wbig_d = nc.dram_tensor("wbig", [128, 1024], dt.float32r, kind="ExternalInput")
    we_d = nc.dram_tensor("we", [128, 18], dt.float32r, kind="ExternalInput")
    wout_d = nc.dram_tensor("woutt", [128, 2], dt.float32r, kind="ExternalInput")
    bbig_d = nc.dram_tensor("bbig", [128, 2], dt.float32, kind="ExternalInput")
    ones2_d = nc.dram_tensor("ones2", [3, 2], dt.float32r, kind="ExternalInput")
    sel_d = nc.dram_tensor("sel", [2, 256], dt.float32r, kind="ExternalInput")
    yout_d = nc.dram_tensor("yout", [nblk, RBLK], dt.float32, kind="ExternalOutput")

    with tile.TileContext(nc) as tc:
        with (
            tc.tile_pool(name="consts", bufs=1) as cpool,
            tc.tile_pool(name="xin", bufs=3) as xpool,
            tc.tile_pool(name="work", bufs=2) as wpool,
            tc.tile_pool(name="ps_e", bufs=1, space="PSUM") as ps_e,
            tc.tile_pool(name="ps_z2", bufs=1, space="PSUM") as ps_z2,
            tc.tile_pool(name="ps_a", bufs=1, space="PSUM") as ps_a,
            tc.tile_pool(name="ps_z", bufs=1, space="PSUM") as ps_z,
            tc.tile_pool(name="ps_y", bufs=1, space="PSUM") as ps_y,
        ):
            wbig_s = cpool.tile([128, 4, 256], dt.float32r)
            nc.sync.dma_start(wbig_s[:], wbig_d[:].rearrange("p (k m) -> p k m", k=4))
            we_s = cpool.tile([128, 6, 3], dt.float32r)
            nc.sync.dma_start(we_s[:], we_d[:].rearrange("p (j c) -> p j c", j=6))
            wout_s = cpool.tile([128, 2], dt.float32r)
            nc.sync.dma_start(wout_s[:], wout_d[:])
            bbig_s = cpool.tile([128, 2], dt.float32)
            nc.sync.dma_start(bbig_s[:], bbig_d[:])
            ones2_s = cpool.tile([3, 2], dt.float32r)
            nc.sync.dma_start(ones2_s[:], ones2_d[:])
            sel_s = cpool.tile([2, 256], dt.float32r)
            nc.sync.dma_start(sel_s[:], sel_d[:])

            xin_ap = xin_d[:].rearrange("(k p) n -> p k n", p=128)

            for i in range(nblk):
                c0 = i * RBLK
                xblk = xpool.tile([128, 8, RBLK], dt.float32r, tag="xblk")
                nc.sync.dma_start(xblk[:], xin_ap[:, :, c0:c0 + RBLK])

                # e.T [3, R] = attention logits per row (cols of lhsT: e1,e2,e3)
                e_ps = ps_e.tile([3, RBLK], dt.float32, tag="e")
                for idx, (j, k) in enumerate([(0, 2), (1, 3), (2, 4),
                                              (3, 5), (4, 6), (5, 7)]):
                    nc.tensor.matmul(e_ps[:], we_s[:, j, :], xblk[:, k, :],
                                     start=(idx == 0), stop=(idx == 5))

                # leaky_relu(x) = max(x, 0.01*x); scaled copy lands in SBUF so
                # the max reads only one PSUM operand (walrus restriction)
                ec = wpool.tile([3, RBLK], dt.float32, tag="ec")
                nc.scalar.activation(ec[:], e_ps[:], act.Copy, scale=0.01)
                el = wpool.tile([3, RBLK], dt.float32, tag="el")
                nc.vector.tensor_tensor(el[:], e_ps[:], ec[:], op=op.max)
                t_s = wpool.tile([3, RBLK], dt.float32r, tag="t")
                nc.scalar.activation(t_s[:], el[:], act.Exp)

                # softmax denominator (x2 rows) via ones-matmul, then 1/Z
                z2_ps = ps_z2.tile([2, RBLK], dt.float32, tag="z2")
                nc.tensor.matmul(z2_ps[:], ones2_s[:], t_s[:],
                                 start=True, stop=True)
                rz = wpool.tile([2, RBLK], dt.float32, tag="rz")
                nc.vector.reciprocal(rz[:], z2_ps[:])
                al = wpool.tile([2, RBLK], dt.float32r, tag="al")
                nc.vector.tensor_tensor(al[:], t_s[0:2, :], rz[:], op=op.mult)

                # replicate a1, a2 across 128 partitions (K=2 selector matmul)
                a1_ps = ps_a.tile([128, RBLK], dt.float32, tag="a1")
                nc.tensor.matmul(a1_ps[:], sel_s[:, 0:128], al[:],
                                 start=True, stop=True)
                a2_ps = ps_a.tile([128, RBLK], dt.float32, tag="a2")
                nc.tensor.matmul(a2_ps[:], sel_s[:, 128:256], al[:],
                                 start=True, stop=True)

                # agg.T = H.T + a1*D1.T + a2*D2.T  (two 128-feature chunks)
                aggs = []
                for f in range(2):
                    m1 = wpool.tile([128, RBLK], dt.float32, tag=f"m1_{f}")
                    nc.vector.tensor_tensor(m1[:], a1_ps[:], xblk[:, 4 + f, :],
                                            op=op.mult)
                    m2 = wpool.tile([128, RBLK], dt.float32, tag=f"m2_{f}")
                    nc.vector.tensor_tensor(m2[:], a2_ps[:], xblk[:, 6 + f, :],
                                            op=op.mult)
                    s1 = wpool.tile([128, RBLK], dt.float32, tag=f"s1_{f}")
                    nc.gpsimd.tensor_tensor(s1[:], m1[:], m2[:], op=op.add)
                    agg = wpool.tile([128, RBLK], dt.float32r, tag=f"agg_{f}")
                    nc.gpsimd.tensor_tensor(agg[:], s1[:], xblk[:, 2 + f, :],
                                            op=op.add)
                    aggs.append(agg)

                # z.T = Wbig @ [I; agg].T, relu(+bias), then y = wout . m
                mres = []
                for m in range(2):
                    z_ps = ps_z.tile([128, RBLK], dt.float32, tag=f"z{m}")
                    for k in range(4):
                        rhs = xblk[:, k, :] if k < 2 else aggs[k - 2][:]
                        nc.tensor.matmul(z_ps[:],
                                         wbig_s[:, k, m * 128:(m + 1) * 128],
                                         rhs, start=(k == 0), stop=(k == 3))
                    mr = wpool.tile([128, RBLK], dt.float32r, tag=f"mres{m}")
                    nc.scalar.activation(mr[:], z_ps[:], act.Relu,
                                         bias=bbig_s[:, m:m + 1])
                    mres.append(mr)

                y_ps = ps_y.tile([1, RBLK], dt.float32, tag="y")
                for m in range(2):
                    nc.tensor.matmul(y_ps[:], wout_s[:, m:m + 1], mres[m][:],
                                     start=(m == 0), stop=(m == 1))
                ys = wpool.tile([1, RBLK], dt.float32, tag="ys")
                nc.scalar.activation(ys[:], y_ps[:], act.Copy)
                nc.sync.dma_start(yout_d[i:i + 1, :], ys[:])

    nc.compile()
    return nc


def _get_nc(rows):
    if rows not in _cache:
        _cache[rows] = _build_nc(rows)
    return _cache[rows]


def _host_prep(indiv_f, hierarchy_f, strength_f, recency_f,
               w_attn1, Wv, bv, Wo, bo, Wfc, bfc, Wout):
    """Fold weights and build the device-side arrays (full, unsharded)."""
    f32 = np.float32
    indiv_f = np.asarray(indiv_f, f32)
    H = np.asarray(hierarchy_f, f32)
    S = np.asarray(strength_f, f32)
    R = np.asarray(recency_f, f32)
    n = indiv_f.shape[0]

    w1 = np.asarray(w_attn1, f32).reshape(-1)
    w1H, w1S = w1[:CF], w1[CF:]
    u = w1H + w1S

    Wv64 = np.asarray(Wv, np.float64)
    Wo64 = np.asarray(Wo, np.float64)
    Wfc64 = np.asarray(Wfc, np.float64)
    Wbig = (Wfc64 @ Wo64 @ Wv64).astype(f32)                    # [256, 512]
    bbig = (Wfc64 @ (Wo64 @ np.asarray(bv, np.float64)
                     + np.asarray(bo, np.float64))
            + np.asarray(bfc, np.float64)).astype(f32)          # [256]
    wout = np.asarray(Wout, f32).reshape(-1)                    # [256]

    X = np.empty((1024, n), f32)
    X[0:256] = indiv_f.T
    X[256:512] = H.T
    X[512:768] = S.T
    X[512:768] -= X[256:512]
    X[768:1024] = R.T
    X[768:1024] -= X[256:512]

    wbig_t = np.ascontiguousarray(
        Wbig.T.reshape(4, 128, 256).transpose(1, 0, 2).reshape(128, 1024))
    we = np.zeros((128, 6, 3), f32)
    we[:, 0, :] = u[0:128, None]
    we[:, 1, :] = u[128:256, None]
    we[:, 2, 0] = w1S[0:128]
    we[:, 3, 0] = w1S[128:256]
    we[:, 4, 1] = w1S[0:128]
    we[:, 5, 1] = w1S[128:256]
    we = we.reshape(128, 18)
    wout_t = np.ascontiguousarray(wout.reshape(2, 128).T)
    bbig2 = np.ascontiguousarray(bbig.reshape(2, 128).T)
    ones2 = np.ones((3, 2), f32)
    sel = np.zeros((2, 256), f32)
    sel[0, 0:128] = 1.0
    sel[1, 128:256] = 1.0
    return X, dict(wbig=wbig_t, we=we, woutt=wout_t, bbig=bbig2,
                   ones2=ones2, sel=sel)


def kernel(indiv_f, hierarchy_f, strength_f, recency_f, season_ids,
           w_attn1, Wq, bq, Wk, bk, Wv, bv, Wo, bo, Wfc, bfc, Wout, bout,
           **_unused):
    from concourse.bass_utils import run_bass_kernel_spmd

    X, consts = _host_prep(indiv_f, hierarchy_f, strength_f, recency_f,
                           w_attn1, Wv, bv, Wo, bo, Wfc, bfc, Wout)

    nc = _get_nc(ROWS)
    in_maps = []
    for c in range(N_CORES):
        m = dict(consts)
        m["xin"] = X[:, c * ROWS:(c + 1) * ROWS]
        in_maps.append(m)
    res = run_bass_kernel_spmd(nc, in_maps, core_ids=list(range(N_CORES)))
    global _last_result
    _last_result = res
    y = np.concatenate([res.results[c]["yout"].reshape(-1)
                        for c in range(N_CORES)])

    ids = np.asarray(season_ids).reshape(-1)
    sums = np.bincount(ids, weights=y.astype(np.float64),
                       minlength=NUM_SEASONS)
    counts = np.bincount(ids, minlength=NUM_SEASONS)
    out = sums / np.maximum(counts, 1) + float(np.asarray(bout).reshape(-1)[0])
    return out.astype(np.float32).reshape(NUM_SEASONS, 1)


# revision 6
# speedup vs baseline: 1.3998x; 1.3998x over previous
"""Trainium2 Bass kernel for nn_Nonhier_Optmatch (8-core SPMD, bf16).

Contract: kernel(**inputs) takes the FULL unsharded inputs and returns the
FULL [8192, 1] float32 output. Internally shards the N=262144 row axis
across 8 NeuronCores.

Math notes (exact reformulations of the reference):
  - The MHA softmax is over a size-1 axis => attn weight == 1.0, so q/k/Wq/Wk
    are dead code and attn_out = (x @ Wv.T + bv) @ Wo.T + bo.
  - Linear folding: member_emb = relu(x @ Wbig.T + bbig) with
    Wbig = Wfc @ Wo @ Wv and bbig = Wfc @ (Wo @ bv + bo) + bfc.
  - Output fold: out[s] = (sum_{i in s} relu(z_i) . wout) / count_s + bout,
    so only the scalar y_i = relu(z_i) . wout leaves the device; the ragged
    per-season mean over scalars is a host bincount.
  - With D1 = S - H, D2 = R - H and u = w1H + w1S:
      e1 = u.H + w1S.D1, e2 = u.H + w1S.D2, e3 = u.H
      agg = H + a1*D1 + a2*D2   (a = softmax(leaky_relu(e)))
  - Division-free softmax: a_c = exp(prelu(e_c) - ln(sum_c exp(prelu(e_c))));
    the -lnZ subtraction rides a PSUM-accumulating matmul, so ACT only needs
    {Prelu, Exp, Ln, Relu, Copy} - all in one HW activation table.

Device layout: feature-major (activations transposed on host) so the feature
contractions run on the PE. Per-row alphas are broadcast across partitions by
a DRAM-bounce DMA (SBUF partition-broadcast is not supported).
"""
import sys

sys.path.insert(0, '/opt/trn_rl_repo')

import numpy as np
import ml_dtypes

N_CORES = 8
N_TOTAL = 262144
ROWS = N_TOTAL // N_CORES        # 32768 rows per core
RBLK = 512                       # rows per block (PSUM bank width in fp32)
CF = 256
EMB = 256
NUM_SEASONS = 8192

_cache = {}


def _build_nc(rows, sim_safe=False):
    import concourse.bacc as bacc
    import concourse.tile as tile
    import concourse.mybir as mybir

    dt = mybir.dt
    op = mybir.AluOpType
    act = mybir.ActivationFunctionType
    bf = dt.bfloat16
    nblk = rows // RBLK

    nc = bacc.Bacc("TRN2", target_bir_lowering=False, debug=False,
                   num_devices=N_CORES)

    xin_d = nc.dram_tensor("xin", [1024, rows], bf, kind="ExternalInput")
    wbig_d = nc.dram_tensor("wbig", [128, 1024], bf, kind="ExternalInput")
    we_d = nc.dram_tensor("we", [128, 18], bf, kind="ExternalInput")
    wout_d = nc.dram_tensor("woutt", [128, 2], bf, kind="ExternalInput")
    bbig_d = nc.dram_tensor("bbig", [128, 2], dt.float32, kind="ExternalInput")
    ones3_d = nc.dram_tensor("ones3", [3, 1], bf, kind="ExternalInput")
    sel12_d = nc.dram_tensor("sel12", [3, 2], bf, kind="ExternalInput")
    neg1_d = nc.dram_tensor("neg1", [1, 2], bf, kind="ExternalInput")
    yout_d = nc.dram_tensor("yout", [nblk, RBLK], dt.float32,
                            kind="ExternalOutput")

    with tile.TileContext(nc) as tc:
        with (
            tc.tile_pool(name="consts", bufs=1) as cpool,
            tc.tile_pool(name="xin", bufs=3) as xpool,
            tc.tile_pool(name="work", bufs=2) as wpool,
            tc.tile_pool(name="dscr", bufs=2, space="DRAM") as dpool,
            tc.tile_pool(name="ps_e", bufs=1, space="PSUM") as ps_e,
            tc.tile_pool(name="ps_z1", bufs=1, space="PSUM") as ps_z1,
            tc.tile_pool(name="ps_el2", bufs=1, space="PSUM") as ps_el2,
            tc.tile_pool(name="ps_z", bufs=2, space="PSUM") as ps_z,
            tc.tile_pool(name="ps_y", bufs=1, space="PSUM") as ps_y,
        ):
            wbig_s = cpool.tile([128, 4, 256], bf)
            nc.sync.dma_start(wbig_s[:], wbig_d[:].rearrange("p (k m) -> p k m", k=4))
            we_s = cpool.tile([128, 6, 3], bf)
            nc.sync.dma_start(we_s[:], we_d[:].rearrange("p (j c) -> p j c", j=6))
            wout_s = cpool.tile([128, 2], bf)
            nc.sync.dma_start(wout_s[:], wout_d[:])
            bbig_s = cpool.tile([128, 2], dt.float32)
            nc.sync.dma_start(bbig_s[:], bbig_d[:])
            ones3_s = cpool.tile([3, 1], bf)
            nc.sync.dma_start(ones3_s[:], ones3_d[:])
            sel12_s = cpool.tile([3, 2], bf)
            nc.sync.dma_start(sel12_s[:], sel12_d[:])
            neg1_s = cpool.tile([1, 2], bf)
            nc.sync.dma_start(neg1_s[:], neg1_d[:])

            xin_ap = xin_d[:].rearrange("(k p) n -> p k n", p=128)

            for i in range(nblk):
                c0 = i * RBLK
                # chunk order: 0,1=I  2,3=H  4,5=D1  6,7=D2
                xblk = xpool.tile([128, 8, RBLK], bf, tag="xblk")
                nc.sync.dma_start(xblk[:], xin_ap[:, :, c0:c0 + RBLK])

                # e.T [3, R]: attention logits (lhsT cols: e1, e2, e3)
                e_ps = ps_e.tile([3, RBLK], dt.float32, tag="e")
                for idx, (j, k) in enumerate([(0, 2), (1, 3), (2, 4),
                                              (3, 5), (4, 6), (5, 7)]):
                    nc.tensor.matmul(e_ps[:], we_s[:, j, :], xblk[:, k, :],
                                     start=(idx == 0), stop=(idx == 5))

                # el = leaky_relu(e)
                el = wpool.tile([3, RBLK], bf, tag="el")
                if sim_safe:
                    ec = wpool.tile([3, RBLK], dt.float32, tag="ec")
                    nc.vector.tensor_scalar_mul(ec[:], e_ps[:], 0.01)
                    nc.vector.tensor_tensor(el[:], e_ps[:], ec[:], op=op.max)
                else:
                    nc.scalar.activation(el[:], e_ps[:], act.Prelu, alpha=0.01)

                # t = exp(el); Z = sum_c t; alpha_c = exp(el_c - lnZ)
                t_s = wpool.tile([3, RBLK], bf, tag="t")
                nc.scalar.activation(t_s[:], el[:], act.Exp)
                z_ps = ps_z1.tile([1, RBLK], dt.float32, tag="z1")
                nc.tensor.matmul(z_ps[:], ones3_s[:], t_s[:],
                                 start=True, stop=True)
                lnz = wpool.tile([1, RBLK], bf, tag="lnz")
                nc.scalar.activation(lnz[:], z_ps[:], act.Ln)
                el2_ps = ps_el2.tile([2, RBLK], dt.float32, tag="el2")
                nc.tensor.matmul(el2_ps[:], sel12_s[:], el[:],
                                 start=True, stop=False)
                nc.tensor.matmul(el2_ps[:], neg1_s[:], lnz[:],
                                 start=False, stop=True)
                al = wpool.tile([2, RBLK], bf, tag="al")
                nc.scalar.activation(al[:], el2_ps[:], act.Exp)

                # broadcast alphas across partitions via DRAM bounce
                scr = dpool.tile([2, RBLK], bf, tag="scr")
                nc.sync.dma_start(scr[:], al[:])
                a12 = wpool.tile([128, 2, RBLK], bf, tag="a12")
                nc.sync.dma_start(
                    a12[:], scr[:].unsqueeze(0).broadcast_to([128, 2, RBLK]))

                # agg.T = H.T + a1*D1.T + a2*D2.T  (both chunks at once)
                a1v = a12[:, 0:1, :].broadcast_to([128, 2, RBLK])
                a2v = a12[:, 1:2, :].broadcast_to([128, 2, RBLK])
                m1 = wpool.tile([128, 2, RBLK], bf, tag="m1")
                nc.vector.tensor_tensor(m1[:], xblk[:, 4:6, :], a1v, op=op.mult)
                m2 = wpool.tile([128, 2, RBLK], bf, tag="m2")
                nc.vector.tensor_tensor(m2[:], xblk[:, 6:8, :], a2v, op=op.mult)
                s1 = wpool.tile([128, 2, RBLK], bf, tag="s1")
                nc.vector.tensor_tensor(s1[:], m1[:], m2[:], op=op.add)
                agg = wpool.tile([128, 2, RBLK], bf, tag="agg")
                nc.gpsimd.tensor_tensor(agg[:], s1[:], xblk[:, 2:4, :],
                                        op=op.add)

                # z.T = Wbig @ [I; agg].T; m = relu(z + bbig); y = wout . m
                z_ps = ps_z.tile([128, 2, RBLK], dt.float32, tag="z")
                for m in range(2):
                    for k in range(4):
                        rhs = xblk[:, k, :] if k < 2 else agg[:, k - 2, :]
                        nc.tensor.matmul(z_ps[:, m, :],
                                         wbig_s[:, k, m * 128:(m + 1) * 128],
                                         rhs, start=(k == 0), stop=(k == 3))
                mres = wpool.tile([128, 2, RBLK], bf, tag="mres")
                for m in range(2):
                    nc.scalar.activation(mres[:, m, :], z_ps[:, m, :], act.Relu,
                                         bias=bbig_s[:, m:m + 1])

                y_ps = ps_y.tile([1, RBLK], dt.float32, tag="y")
                for m in range(2):
                    nc.tensor.matmul(y_ps[:], wout_s[:, m:m + 1], mres[:, m, :],
                                     start=(m == 0), stop=(m == 1))
                ys = wpool.tile([1, RBLK], dt.float32, tag="ys")
                nc.vector.tensor_copy(ys[:], y_ps[:])
                nc.sync.dma_start(yout_d[i:i + 1, :], ys[:])

    nc.compile()
    return nc


def _get_nc(rows):
    if rows not in _cache:
        _cache[rows] = _build_nc(rows)
    return _cache[rows]


def _host_prep(indiv_f, hierarchy_f, strength_f, recency_f,
               w_attn1, Wv, bv, Wo, bo, Wfc, bfc, Wout):
    """Fold weights and build the device-side arrays (full, unsharded)."""
    f32 = np.float32
    bf16 = ml_dtypes.bfloat16
    indiv_f = np.asarray(indiv_f, f32)
    H = np.asarray(hierarchy_f, f32)
    S = np.asarray(strength_f, f32)
    R = np.asarray(recency_f, f32)
    n = indiv_f.shape[0]

    w1 = np.asarray(w_attn1, f32).reshape(-1)
    w1H, w1S = w1[:CF], w1[CF:]
    u = w1H + w1S

    Wv64 = np.asarray(Wv, np.float64)
    Wo64 = np.asarray(Wo, np.float64)
    Wfc64 = np.asarray(Wfc, np.float64)
    Wbig = (Wfc64 @ Wo64 @ Wv64).astype(f32)                    # [256, 512]
    bbig = (Wfc64 @ (Wo64 @ np.asarray(bv, np.float64)
                     + np.asarray(bo, np.float64))
            + np.asarray(bfc, np.float64)).astype(f32)          # [256]
    wout = np.asarray(Wout, f32).reshape(-1)                    # [256]

    X = np.empty((1024, n), bf16)
    X[0:256] = indiv_f.T
    X[256:512] = H.T
    X[512:768] = (S - H).T
    X[768:1024] = (R - H).T

    wbig_t = np.ascontiguousarray(
        Wbig.T.reshape(4, 128, 256).transpose(1, 0, 2).reshape(128, 1024)
    ).astype(bf16)
    we = np.zeros((128, 6, 3), f32)
    we[:, 0, :] = u[0:128, None]
    we[:, 1, :] = u[128:256, None]
    we[:, 2, 0] = w1S[0:128]
    we[:, 3, 0] = w1S[128:256]
    we[:, 4, 1] = w1S[0:128]
    we[:, 5, 1] = w1S[128:256]
    we = we.reshape(128, 18).astype(bf16)
    wout_t = np.ascontiguousarray(wout.reshape(2, 128).T).astype(bf16)
    bbig2 = np.ascontiguousarray(bbig.reshape(2, 128).T)
    ones3 = np.ones((3, 1), bf16)
    sel12 = np.zeros((3, 2), np.float32)
    sel12[0, 0] = 1.0
    sel12[1, 1] = 1.0
    neg1 = -np.ones((1, 2), np.float32)
    return X, dict(wbig=wbig_t, we=we, woutt=wout_t, bbig=bbig2,
                   ones3=ones3, sel12=sel12.astype(bf16),
                   neg1=neg1.astype(bf16))


_last_result = None


def kernel(indiv_f, hierarchy_f, strength_f, recency_f, season_ids,
           w_attn1, Wq, bq, Wk, bk, Wv, bv, Wo, bo, Wfc, bfc, Wout, bout,
           **_unused):
    from concourse.bass_utils import run_bass_kernel_spmd

    X, consts = _host_prep(indiv_f, hierarchy_f, strength_f, recency_f,
                           w_attn1, Wv, bv, Wo, bo, Wfc, bfc, Wout)

    nc = _get_nc(ROWS)
    in_maps = []
    for c in range(N_CORES):
        m = dict(consts)
        m["xin"] = X[:, c * ROWS:(c + 1) * ROWS]
        in_maps.append(m)
    res = run_bass_kernel_spmd(nc, in_maps, core_ids=list(range(N_CORES)))
    global _last_result
    _last_result = res
    y = np.concatenate([res.results[c]["yout"].reshape(-1)
                        for c in range(N_CORES)])

    ids = np.asarray(season_ids).reshape(-1)
    sums = np.bincount(ids, weights=y.astype(np.float64),
                       minlength=NUM_SEASONS)
    counts = np.bincount(ids, minlength=NUM_SEASONS)
    out = sums / np.maximum(counts, 1) + float(np.asarray(bout).reshape(-1)[0])
    return out.astype(np.float32).reshape(NUM_SEASONS, 1)


# revision 7
# speedup vs baseline: 1.4839x; 1.0601x over previous
"""Trainium2 Bass kernel for nn_Nonhier_Optmatch (8-core SPMD, bf16).

Contract: kernel(**inputs) takes the FULL unsharded inputs and returns the
FULL [8192, 1] float32 output. Internally shards the N=262144 row axis
across 8 NeuronCores.

Math notes (exact reformulations of the reference):
  - The MHA softmax is over a size-1 axis => attn weight == 1.0, so q/k/Wq/Wk
    are dead code and attn_out = (x @ Wv.T + bv) @ Wo.T + bo.
  - Linear folding: member_emb = relu(x @ Wbig.T + bbig) with
    Wbig = Wfc @ Wo @ Wv and bbig = Wfc @ (Wo @ bv + bo) + bfc.
  - Output fold: out[s] = (sum_{i in s} relu(z_i) . wout) / count_s + bout,
    so only the scalar y_i = relu(z_i) . wout leaves the device; the ragged
    per-season mean over scalars is a host bincount.
  - With D1 = S - H, D2 = R - H and u = w1H + w1S:
      e1 = u.H + w1S.D1, e2 = u.H + w1S.D2, e3 = u.H
      agg = H + a1*D1 + a2*D2   (a = softmax(leaky_relu(e)))
  - Division-free softmax: a_c = exp(prelu(e_c) - ln(sum_c exp(prelu(e_c))));
    the -lnZ subtraction rides a PSUM-accumulating matmul, so ACT only needs
    {Prelu, Exp, Ln, Relu, Copy} - all in one HW activation table.

Device layout: feature-major (activations transposed on host) so the feature
contractions run on the PE. Per-row alphas are broadcast across partitions by
a DRAM-bounce DMA (SBUF partition-broadcast is not supported).
"""
import sys

sys.path.insert(0, '/opt/trn_rl_repo')

import numpy as np
import ml_dtypes

N_CORES = 8
N_TOTAL = 262144
ROWS = N_TOTAL // N_CORES        # 32768 rows per core
RBLK = 512                       # rows per block (PSUM bank width in fp32)
CF = 256
EMB = 256
NUM_SEASONS = 8192

_cache = {}


def _force_single_act_table(bacc):
    """Pin all activations to natural_log_exp_and_others (it contains every
    func this kernel uses: Prelu/Exp/Ln/Relu/Copy). The default chooser picks
    the first table per func, which alternates exp_and_others <-> natural_log
    and costs 2x 1.3us ACT_TABLE_LOAD per block."""
    from concourse.hw_specs import get_activation_tables as _real
    import functools

    @functools.cache
    def _only(arch):
        tabs = _real(arch)
        return {name: (s if name == "natural_log_exp_and_others" else set())
                for name, s in tabs.items()}

    bacc.get_activation_tables = _only


def _build_nc(rows, sim_safe=False):
    import concourse.bacc as bacc
    import concourse.tile as tile
    import concourse.mybir as mybir

    if not sim_safe:
        _force_single_act_table(bacc)

    dt = mybir.dt
    op = mybir.AluOpType
    act = mybir.ActivationFunctionType
    bf = dt.bfloat16
    nblk = rows // RBLK

    nc = bacc.Bacc("TRN2", target_bir_lowering=False, debug=False,
                   num_devices=N_CORES)

    xin_d = nc.dram_tensor("xin", [1024, rows], bf, kind="ExternalInput")
    wbig_d = nc.dram_tensor("wbig", [128, 1024], bf, kind="ExternalInput")
    we_d = nc.dram_tensor("we", [128, 18], bf, kind="ExternalInput")
    wout_d = nc.dram_tensor("woutt", [128, 2], bf, kind="ExternalInput")
    bbig_d = nc.dram_tensor("bbig", [128, 2], dt.float32, kind="ExternalInput")
    ones3_d = nc.dram_tensor("ones3", [3, 1], bf, kind="ExternalInput")
    sel12_d = nc.dram_tensor("sel12", [3, 2], bf, kind="ExternalInput")
    neg1_d = nc.dram_tensor("neg1", [1, 2], bf, kind="ExternalInput")
    yout_d = nc.dram_tensor("yout", [nblk, RBLK], dt.float32,
                            kind="ExternalOutput")

    with tile.TileContext(nc) as tc:
        with (
            tc.tile_pool(name="consts", bufs=1) as cpool,
            tc.tile_pool(name="xin", bufs=3) as xpool,
            tc.tile_pool(name="work", bufs=2) as wpool,
            tc.tile_pool(name="dscr", bufs=2, space="DRAM") as dpool,
            tc.tile_pool(name="ps_e", bufs=1, space="PSUM") as ps_e,
            tc.tile_pool(name="ps_z1", bufs=1, space="PSUM") as ps_z1,
            tc.tile_pool(name="ps_el2", bufs=1, space="PSUM") as ps_el2,
            tc.tile_pool(name="ps_z", bufs=2, space="PSUM") as ps_z,
            tc.tile_pool(name="ps_y", bufs=1, space="PSUM") as ps_y,
        ):
            wbig_s = cpool.tile([128, 4, 256], bf)
            nc.sync.dma_start(wbig_s[:], wbig_d[:].rearrange("p (k m) -> p k m", k=4))
            we_s = cpool.tile([128, 6, 3], bf)
            nc.sync.dma_start(we_s[:], we_d[:].rearrange("p (j c) -> p j c", j=6))
            wout_s = cpool.tile([128, 2], bf)
            nc.sync.dma_start(wout_s[:], wout_d[:])
            bbig_s = cpool.tile([128, 2], dt.float32)
            nc.sync.dma_start(bbig_s[:], bbig_d[:])
            ones3_s = cpool.tile([3, 1], bf)
            nc.sync.dma_start(ones3_s[:], ones3_d[:])
            sel12_s = cpool.tile([3, 2], bf)
            nc.sync.dma_start(sel12_s[:], sel12_d[:])
            neg1_s = cpool.tile([1, 2], bf)
            nc.sync.dma_start(neg1_s[:], neg1_d[:])

            xin_ap = xin_d[:].rearrange("(k p) n -> p k n", p=128)

            for i in range(nblk):
                c0 = i * RBLK
                # chunk order: 0,1=I  2,3=H  4,5=D1  6,7=D2
                xblk = xpool.tile([128, 8, RBLK], bf, tag="xblk")
                nc.sync.dma_start(xblk[:], xin_ap[:, :, c0:c0 + RBLK])

                # e.T [3, R]: attention logits (lhsT cols: e1, e2, e3)
                e_ps = ps_e.tile([3, RBLK], dt.float32, tag="e")
                for idx, (j, k) in enumerate([(0, 2), (1, 3), (2, 4),
                                              (3, 5), (4, 6), (5, 7)]):
                    nc.tensor.matmul(e_ps[:], we_s[:, j, :], xblk[:, k, :],
                                     start=(idx == 0), stop=(idx == 5))

                # el = leaky_relu(e)
                el = wpool.tile([3, RBLK], bf, tag="el")
                if sim_safe:
                    ec = wpool.tile([3, RBLK], dt.float32, tag="ec")
                    nc.vector.tensor_scalar_mul(ec[:], e_ps[:], 0.01)
                    nc.vector.tensor_tensor(el[:], e_ps[:], ec[:], op=op.max)
                else:
                    nc.scalar.activation(el[:], e_ps[:], act.Prelu, alpha=0.01)

                # t = exp(el); Z = sum_c t; alpha_c = exp(el_c - lnZ)
                t_s = wpool.tile([3, RBLK], bf, tag="t")
                nc.scalar.activation(t_s[:], el[:], act.Exp)
                z_ps = ps_z1.tile([1, RBLK], dt.float32, tag="z1")
                nc.tensor.matmul(z_ps[:], ones3_s[:], t_s[:],
                                 start=True, stop=True)
                lnz = wpool.tile([1, RBLK], bf, tag="lnz")
                nc.scalar.activation(lnz[:], z_ps[:], act.Ln)
                el2_ps = ps_el2.tile([2, RBLK], dt.float32, tag="el2")
                nc.tensor.matmul(el2_ps[:], sel12_s[:], el[:],
                                 start=True, stop=False)
                nc.tensor.matmul(el2_ps[:], neg1_s[:], lnz[:],
                                 start=False, stop=True)
                al = wpool.tile([2, RBLK], bf, tag="al")
                nc.scalar.activation(al[:], el2_ps[:], act.Exp)

                # broadcast alphas across partitions via DRAM bounce
                scr = dpool.tile([2, RBLK], bf, tag="scr")
                nc.sync.dma_start(scr[:], al[:])
                a12 = wpool.tile([128, 2, RBLK], bf, tag="a12")
                nc.sync.dma_start(
                    a12[:], scr[:].unsqueeze(0).broadcast_to([128, 2, RBLK]))

                # agg.T = H.T + a1*D1.T + a2*D2.T  (both chunks at once)
                a1v = a12[:, 0:1, :].broadcast_to([128, 2, RBLK])
                a2v = a12[:, 1:2, :].broadcast_to([128, 2, RBLK])
                m1 = wpool.tile([128, 2, RBLK], bf, tag="m1")
                nc.vector.tensor_tensor(m1[:], xblk[:, 4:6, :], a1v, op=op.mult)
                m2 = wpool.tile([128, 2, RBLK], bf, tag="m2")
                nc.vector.tensor_tensor(m2[:], xblk[:, 6:8, :], a2v, op=op.mult)
                s1 = wpool.tile([128, 2, RBLK], bf, tag="s1")
                nc.vector.tensor_tensor(s1[:], m1[:], m2[:], op=op.add)
                agg = wpool.tile([128, 2, RBLK], bf, tag="agg")
                nc.gpsimd.tensor_tensor(agg[:], s1[:], xblk[:, 2:4, :],
                                        op=op.add)

                # z.T = Wbig @ [I; agg].T; m = relu(z + bbig); y = wout . m
                z_ps = ps_z.tile([128, 2, RBLK], dt.float32, tag="z")
                for m in range(2):
                    for k in range(4):
                        rhs = xblk[:, k, :] if k < 2 else agg[:, k - 2, :]
                        nc.tensor.matmul(z_ps[:, m, :],
                                         wbig_s[:, k, m * 128:(m + 1) * 128],
                                         rhs, start=(k == 0), stop=(k == 3))
                mres = wpool.tile([128, 2, RBLK], bf, tag="mres")
                for m in range(2):
                    nc.scalar.activation(mres[:, m, :], z_ps[:, m, :], act.Relu,
                                         bias=bbig_s[:, m:m + 1])

                y_ps = ps_y.tile([1, RBLK], dt.float32, tag="y")
                for m in range(2):
                    nc.tensor.matmul(y_ps[:], wout_s[:, m:m + 1], mres[:, m, :],
                                     start=(m == 0), stop=(m == 1))
                ys = wpool.tile([1, RBLK], dt.float32, tag="ys")
                nc.vector.tensor_copy(ys[:], y_ps[:])
                nc.sync.dma_start(yout_d[i:i + 1, :], ys[:])

    nc.compile()
    return nc


def _get_nc(rows):
    if rows not in _cache:
        _cache[rows] = _build_nc(rows)
    return _cache[rows]


def _host_prep(indiv_f, hierarchy_f, strength_f, recency_f,
               w_attn1, Wv, bv, Wo, bo, Wfc, bfc, Wout):
    """Fold weights and build the device-side arrays (full, unsharded)."""
    f32 = np.float32
    bf16 = ml_dtypes.bfloat16
    indiv_f = np.asarray(indiv_f, f32)
    H = np.asarray(hierarchy_f, f32)
    S = np.asarray(strength_f, f32)
    R = np.asarray(recency_f, f32)
    n = indiv_f.shape[0]

    w1 = np.asarray(w_attn1, f32).reshape(-1)
    w1H, w1S = w1[:CF], w1[CF:]
    u = w1H + w1S

    Wv64 = np.asarray(Wv, np.float64)
    Wo64 = np.asarray(Wo, np.float64)
    Wfc64 = np.asarray(Wfc, np.float64)
    Wbig = (Wfc64 @ Wo64 @ Wv64).astype(f32)                    # [256, 512]
    bbig = (Wfc64 @ (Wo64 @ np.asarray(bv, np.float64)
                     + np.asarray(bo, np.float64))
            + np.asarray(bfc, np.float64)).astype(f32)          # [256]
    wout = np.asarray(Wout, f32).reshape(-1)                    # [256]

    X = np.empty((1024, n), bf16)
    X[0:256] = indiv_f.T
    X[256:512] = H.T
    X[512:768] = (S - H).T
    X[768:1024] = (R - H).T

    wbig_t = np.ascontiguousarray(
        Wbig.T.reshape(4, 128, 256).transpose(1, 0, 2).reshape(128, 1024)
    ).astype(bf16)
    we = np.zeros((128, 6, 3), f32)
    we[:, 0, :] = u[0:128, None]
    we[:, 1, :] = u[128:256, None]
    we[:, 2, 0] = w1S[0:128]
    we[:, 3, 0] = w1S[128:256]
    we[:, 4, 1] = w1S[0:128]
    we[:, 5, 1] = w1S[128:256]
    we = we.reshape(128, 18).astype(bf16)
    wout_t = np.ascontiguousarray(wout.reshape(2, 128).T).astype(bf16)
    bbig2 = np.ascontiguousarray(bbig.reshape(2, 128).T)
    ones3 = np.ones((3, 1), bf16)
    sel12 = np.zeros((3, 2), np.float32)
    sel12[0, 0] = 1.0
    sel12[1, 1] = 1.0
    neg1 = -np.ones((1, 2), np.float32)
    return X, dict(wbig=wbig_t, we=we, woutt=wout_t, bbig=bbig2,
                   ones3=ones3, sel12=sel12.astype(bf16),
                   neg1=neg1.astype(bf16))


_last_result = None


def kernel(indiv_f, hierarchy_f, strength_f, recency_f, season_ids,
           w_attn1, Wq, bq, Wk, bk, Wv, bv, Wo, bo, Wfc, bfc, Wout, bout,
           **_unused):
    from concourse.bass_utils import run_bass_kernel_spmd

    X, consts = _host_prep(indiv_f, hierarchy_f, strength_f, recency_f,
                           w_attn1, Wv, bv, Wo, bo, Wfc, bfc, Wout)

    nc = _get_nc(ROWS)
    in_maps = []
    for c in range(N_CORES):
        m = dict(consts)
        m["xin"] = X[:, c * ROWS:(c + 1) * ROWS]
        in_maps.append(m)
    res = run_bass_kernel_spmd(nc, in_maps, core_ids=list(range(N_CORES)))
    global _last_result
    _last_result = res
    y = np.concatenate([res.results[c]["yout"].reshape(-1)
                        for c in range(N_CORES)])

    ids = np.asarray(season_ids).reshape(-1)
    sums = np.bincount(ids, weights=y.astype(np.float64),
                       minlength=NUM_SEASONS)
    counts = np.bincount(ids, minlength=NUM_SEASONS)
    out = sums / np.maximum(counts, 1) + float(np.asarray(bout).reshape(-1)[0])
    return out.astype(np.float32).reshape(NUM_SEASONS, 1)


# revision 8
# speedup vs baseline: 1.4999x; 1.0108x over previous
"""Trainium2 Bass kernel for nn_Nonhier_Optmatch (8-core SPMD, bf16).

Contract: kernel(**inputs) takes the FULL unsharded inputs and returns the
FULL [8192, 1] float32 output. Internally shards the N=262144 row axis
across 8 NeuronCores.

Math notes (exact reformulations of the reference):
  - The MHA softmax is over a size-1 axis => attn weight == 1.0, so q/k/Wq/Wk
    are dead code and attn_out = (x @ Wv.T + bv) @ Wo.T + bo.
  - Linear folding: member_emb = relu(x @ Wbig.T + bbig) with
    Wbig = Wfc @ Wo @ Wv and bbig = Wfc @ (Wo @ bv + bo) + bfc.
  - Output fold: out[s] = (sum_{i in s} relu(z_i) . wout) / count_s + bout,
    so only the scalar y_i = relu(z_i) . wout leaves the device; the ragged
    per-season mean over scalars is a host bincount.
  - With D1 = S - H, D2 = R - H and u = w1H + w1S:
      e1 = u.H + w1S.D1, e2 = u.H + w1S.D2, e3 = u.H
      agg = H + a1*D1 + a2*D2   (a = softmax(leaky_relu(e)))
  - Division-free softmax: a_c = exp(prelu(e_c) - ln(sum_c exp(prelu(e_c))));
    the -lnZ subtraction rides a PSUM-accumulating matmul, so ACT only needs
    {Prelu, Exp, Ln, Relu, Copy} - all in one HW activation table.

Device layout: feature-major (activations transposed on host) so the feature
contractions run on the PE. Per-row alphas are broadcast across partitions by
a DRAM-bounce DMA (SBUF partition-broadcast is not supported).
"""
import sys

sys.path.insert(0, '/opt/trn_rl_repo')

import numpy as np
import ml_dtypes

N_CORES = 8
N_TOTAL = 262144
ROWS = N_TOTAL // N_CORES        # 32768 rows per core
RBLK = 512                       # rows per block (PSUM bank width in fp32)
CF = 256
EMB = 256
NUM_SEASONS = 8192

_cache = {}


def _force_single_act_table(bacc):
    """Pin all activations to natural_log_exp_and_others (it contains every
    func this kernel uses: Prelu/Exp/Ln/Relu/Copy). The default chooser picks
    the first table per func, which alternates exp_and_others <-> natural_log
    and costs 2x 1.3us ACT_TABLE_LOAD per block."""
    from concourse.hw_specs import get_activation_tables as _real
    import functools

    @functools.cache
    def _only(arch):
        tabs = _real(arch)
        return {name: (s if name == "natural_log_exp_and_others" else set())
                for name, s in tabs.items()}

    bacc.get_activation_tables = _only


def _build_nc(rows, sim_safe=False):
    import concourse.bacc as bacc
    import concourse.tile as tile
    import concourse.mybir as mybir

    if not sim_safe:
        _force_single_act_table(bacc)

    dt = mybir.dt
    op = mybir.AluOpType
    act = mybir.ActivationFunctionType
    bf = dt.bfloat16
    nblk = rows // RBLK

    nc = bacc.Bacc("TRN2", target_bir_lowering=False, debug=False,
                   num_devices=N_CORES)

    xin_d = nc.dram_tensor("xin", [1024, rows], bf, kind="ExternalInput")
    wbig_d = nc.dram_tensor("wbig", [128, 1024], bf, kind="ExternalInput")
    we_d = nc.dram_tensor("we", [128, 18], bf, kind="ExternalInput")
    wout_d = nc.dram_tensor("woutt", [128, 2], bf, kind="ExternalInput")
    bbig_d = nc.dram_tensor("bbig", [128, 2], dt.float32, kind="ExternalInput")
    ones3_d = nc.dram_tensor("ones3", [3, 1], bf, kind="ExternalInput")
    sel12_d = nc.dram_tensor("sel12", [3, 2], bf, kind="ExternalInput")
    neg1_d = nc.dram_tensor("neg1", [1, 2], bf, kind="ExternalInput")
    yout_d = nc.dram_tensor("yout", [nblk, RBLK], dt.float32,
                            kind="ExternalOutput")

    with tile.TileContext(nc) as tc:
        with (
            tc.tile_pool(name="consts", bufs=1) as cpool,
            tc.tile_pool(name="xin", bufs=6) as xpool,
            tc.tile_pool(name="work", bufs=2) as wpool,
            tc.tile_pool(name="dscr", bufs=2, space="DRAM") as dpool,
            tc.tile_pool(name="ps_e", bufs=1, space="PSUM") as ps_e,
            tc.tile_pool(name="ps_z1", bufs=1, space="PSUM") as ps_z1,
            tc.tile_pool(name="ps_el2", bufs=1, space="PSUM") as ps_el2,
            tc.tile_pool(name="ps_z", bufs=2, space="PSUM") as ps_z,
            tc.tile_pool(name="ps_y", bufs=1, space="PSUM") as ps_y,
        ):
            wbig_s = cpool.tile([128, 4, 256], bf)
            nc.sync.dma_start(wbig_s[:], wbig_d[:].rearrange("p (k m) -> p k m", k=4))
            we_s = cpool.tile([128, 6, 3], bf)
            nc.sync.dma_start(we_s[:], we_d[:].rearrange("p (j c) -> p j c", j=6))
            wout_s = cpool.tile([128, 2], bf)
            nc.sync.dma_start(wout_s[:], wout_d[:])
            bbig_s = cpool.tile([128, 2], dt.float32)
            nc.sync.dma_start(bbig_s[:], bbig_d[:])
            ones3_s = cpool.tile([3, 1], bf)
            nc.sync.dma_start(ones3_s[:], ones3_d[:])
            sel12_s = cpool.tile([3, 2], bf)
            nc.sync.dma_start(sel12_s[:], sel12_d[:])
            neg1_s = cpool.tile([1, 2], bf)
            nc.sync.dma_start(neg1_s[:], neg1_d[:])

            xin_ap = xin_d[:].rearrange("(k p) n -> p k n", p=128)

            for i in range(nblk):
                c0 = i * RBLK
                # chunk order: 0,1=I  2,3=H  4,5=D1  6,7=D2
                xblk = xpool.tile([128, 8, RBLK], bf, tag="xblk")
                nc.sync.dma_start(xblk[:], xin_ap[:, :, c0:c0 + RBLK])

                # e.T [3, R]: attention logits (lhsT cols: e1, e2, e3)
                e_ps = ps_e.tile([3, RBLK], dt.float32, tag="e")
                for idx, (j, k) in enumerate([(0, 2), (1, 3), (2, 4),
                                              (3, 5), (4, 6), (5, 7)]):
                    nc.tensor.matmul(e_ps[:], we_s[:, j, :], xblk[:, k, :],
                                     start=(idx == 0), stop=(idx == 5))

                # el = leaky_relu(e)
                el = wpool.tile([3, RBLK], bf, tag="el")
                if sim_safe:
                    ec = wpool.tile([3, RBLK], dt.float32, tag="ec")
                    nc.vector.tensor_scalar_mul(ec[:], e_ps[:], 0.01)
                    nc.vector.tensor_tensor(el[:], e_ps[:], ec[:], op=op.max)
                else:
                    nc.scalar.activation(el[:], e_ps[:], act.Prelu, alpha=0.01)

                # t = exp(el); Z = sum_c t; alpha_c = exp(el_c - lnZ)
                t_s = wpool.tile([3, RBLK], bf, tag="t")
                nc.scalar.activation(t_s[:], el[:], act.Exp)
                z_ps = ps_z1.tile([1, RBLK], dt.float32, tag="z1")
                nc.tensor.matmul(z_ps[:], ones3_s[:], t_s[:],
                                 start=True, stop=True)
                lnz = wpool.tile([1, RBLK], bf, tag="lnz")
                nc.scalar.activation(lnz[:], z_ps[:], act.Ln)
                el2_ps = ps_el2.tile([2, RBLK], dt.float32, tag="el2")
                nc.tensor.matmul(el2_ps[:], sel12_s[:], el[:],
                                 start=True, stop=False)
                nc.tensor.matmul(el2_ps[:], neg1_s[:], lnz[:],
                                 start=False, stop=True)
                al = wpool.tile([2, RBLK], bf, tag="al")
                nc.scalar.activation(al[:], el2_ps[:], act.Exp)

                # broadcast alphas across partitions via DRAM bounce
                scr = dpool.tile([2, RBLK], bf, tag="scr")
                nc.scalar.dma_start(scr[:], al[:])
                a12 = wpool.tile([128, 2, RBLK], bf, tag="a12")
                nc.scalar.dma_start(
                    a12[:], scr[:].unsqueeze(0).broadcast_to([128, 2, RBLK]))

                # agg.T = H.T + a1*D1.T + a2*D2.T  (both chunks at once)
                a1v = a12[:, 0:1, :].broadcast_to([128, 2, RBLK])
                a2v = a12[:, 1:2, :].broadcast_to([128, 2, RBLK])
                m1 = wpool.tile([128, 2, RBLK], bf, tag="m1")
                nc.vector.tensor_tensor(m1[:], xblk[:, 4:6, :], a1v, op=op.mult)
                m2 = wpool.tile([128, 2, RBLK], bf, tag="m2")
                nc.vector.tensor_tensor(m2[:], xblk[:, 6:8, :], a2v, op=op.mult)
                s1 = wpool.tile([128, 2, RBLK], bf, tag="s1")
                nc.vector.tensor_tensor(s1[:], m1[:], m2[:], op=op.add)
                agg = wpool.tile([128, 2, RBLK], bf, tag="agg")
                nc.gpsimd.tensor_tensor(agg[:], s1[:], xblk[:, 2:4, :],
                                        op=op.add)

                # z.T = Wbig @ [I; agg].T; m = relu(z + bbig); y = wout . m
                z_ps = ps_z.tile([128, 2, RBLK], dt.float32, tag="z")
                for m in range(2):
                    for k in range(4):
                        rhs = xblk[:, k, :] if k < 2 else agg[:, k - 2, :]
                        nc.tensor.matmul(z_ps[:, m, :],
                                         wbig_s[:, k, m * 128:(m + 1) * 128],
                                         rhs, start=(k == 0), stop=(k == 3))
                mres = wpool.tile([128, 2, RBLK], bf, tag="mres")
                for m in range(2):
                    nc.scalar.activation(mres[:, m, :], z_ps[:, m, :], act.Relu,
                                         bias=bbig_s[:, m:m + 1])

                y_ps = ps_y.tile([1, RBLK], dt.float32, tag="y")
                for m in range(2):
                    nc.tensor.matmul(y_ps[:], wout_s[:, m:m + 1], mres[:, m, :],
                                     start=(m == 0), stop=(m == 1))
                ys = wpool.tile([1, RBLK], dt.float32, tag="ys")
                nc.vector.tensor_copy(ys[:], y_ps[:])
                nc.scalar.dma_start(yout_d[i:i + 1, :], ys[:])

    nc.compile()
    return nc


def _get_nc(rows):
    if rows not in _cache:
        _cache[rows] = _build_nc(rows)
    return _cache[rows]


def _host_prep(indiv_f, hierarchy_f, strength_f, recency_f,
               w_attn1, Wv, bv, Wo, bo, Wfc, bfc, Wout):
    """Fold weights and build the device-side arrays (full, unsharded)."""
    f32 = np.float32
    bf16 = ml_dtypes.bfloat16
    indiv_f = np.asarray(indiv_f, f32)
    H = np.asarray(hierarchy_f, f32)
    S = np.asarray(strength_f, f32)
    R = np.asarray(recency_f, f32)
    n = indiv_f.shape[0]

    w1 = np.asarray(w_attn1, f32).reshape(-1)
    w1H, w1S = w1[:CF], w1[CF:]
    u = w1H + w1S

    Wv64 = np.asarray(Wv, np.float64)
    Wo64 = np.asarray(Wo, np.float64)
    Wfc64 = np.asarray(Wfc, np.float64)
    Wbig = (Wfc64 @ Wo64 @ Wv64).astype(f32)                    # [256, 512]
    bbig = (Wfc64 @ (Wo64 @ np.asarray(bv, np.float64)
                     + np.asarray(bo, np.float64))
            + np.asarray(bfc, np.float64)).astype(f32)          # [256]
    wout = np.asarray(Wout, f32).reshape(-1)                    # [256]

    X = np.empty((1024, n), bf16)
    X[0:256] = indiv_f.T
    X[256:512] = H.T
    X[512:768] = (S - H).T
    X[768:1024] = (R - H).T

    wbig_t = np.ascontiguousarray(
        Wbig.T.reshape(4, 128, 256).transpose(1, 0, 2).reshape(128, 1024)
    ).astype(bf16)
    we = np.zeros((128, 6, 3), f32)
    we[:, 0, :] = u[0:128, None]
    we[:, 1, :] = u[128:256, None]
    we[:, 2, 0] = w1S[0:128]
    we[:, 3, 0] = w1S[128:256]
    we[:, 4, 1] = w1S[0:128]
    we[:, 5, 1] = w1S[128:256]
    we = we.reshape(128, 18).astype(bf16)
    wout_t = np.ascontiguousarray(wout.reshape(2, 128).T).astype(bf16)
    bbig2 = np.ascontiguousarray(bbig.reshape(2, 128).T)
    ones3 = np.ones((3, 1), bf16)
    sel12 = np.zeros((3, 2), np.float32)
    sel12[0, 0] = 1.0
    sel12[1, 1] = 1.0
    neg1 = -np.ones((1, 2), np.float32)
    return X, dict(wbig=wbig_t, we=we, woutt=wout_t, bbig=bbig2,
                   ones3=ones3, sel12=sel12.astype(bf16),
                   neg1=neg1.astype(bf16))


_last_result = None


def kernel(indiv_f, hierarchy_f, strength_f, recency_f, season_ids,
           w_attn1, Wq, bq, Wk, bk, Wv, bv, Wo, bo, Wfc, bfc, Wout, bout,
           **_unused):
    from concourse.bass_utils import run_bass_kernel_spmd

    X, consts = _host_prep(indiv_f, hierarchy_f, strength_f, recency_f,
                           w_attn1, Wv, bv, Wo, bo, Wfc, bfc, Wout)

    nc = _get_nc(ROWS)
    in_maps = []
    for c in range(N_CORES):
        m = dict(consts)
        m["xin"] = X[:, c * ROWS:(c + 1) * ROWS]
        in_maps.append(m)
    res = run_bass_kernel_spmd(nc, in_maps, core_ids=list(range(N_CORES)))
    global _last_result
    _last_result = res
    y = np.concatenate([res.results[c]["yout"].reshape(-1)
                        for c in range(N_CORES)])

    ids = np.asarray(season_ids).reshape(-1)
    sums = np.bincount(ids, weights=y.astype(np.float64),
                       minlength=NUM_SEASONS)
    counts = np.bincount(ids, minlength=NUM_SEASONS)
    out = sums / np.maximum(counts, 1) + float(np.asarray(bout).reshape(-1)[0])
    return out.astype(np.float32).reshape(NUM_SEASONS, 1)


# revision 11
# speedup vs baseline: 1.6532x; 1.1022x over previous
"""Trainium2 Bass kernel for nn_Nonhier_Optmatch (8-core SPMD, bf16).

Contract: kernel(**inputs) takes the FULL unsharded inputs and returns the
FULL [8192, 1] float32 output. Internally shards the N=262144 row axis
across 8 NeuronCores.

Math notes (exact reformulations of the reference):
  - The MHA softmax is over a size-1 axis => attn weight == 1.0, so q/k/Wq/Wk
    are dead code and attn_out = (x @ Wv.T + bv) @ Wo.T + bo.
  - Linear folding: member_emb = relu(x @ Wbig.T + bbig) with
    Wbig = Wfc @ Wo @ Wv and bbig = Wfc @ (Wo @ bv + bo) + bfc.
  - Output fold: out[s] = (sum_{i in s} relu(z_i) . wout) / count_s + bout,
    so only the scalar y_i = relu(z_i) . wout leaves the device; the ragged
    per-season mean over scalars is a host bincount.
  - With D1 = S - H, D2 = R - H and u = w1H + w1S:
      e1 = u.H + w1S.D1, e2 = u.H + w1S.D2, e3 = u.H
      agg = H + a1*D1 + a2*D2   (a = softmax(leaky_relu(e)))
  - Division-free softmax: a_c = exp(prelu(e_c) - ln(sum_c exp(prelu(e_c))));
    the -lnZ subtraction rides a PSUM-accumulating matmul, so ACT only needs
    {Prelu, Exp, Ln, Relu, Copy} - all in one HW activation table.

Device layout: feature-major (activations transposed on host) so the feature
contractions run on the PE. Per-row alphas are broadcast across partitions by
a DRAM-bounce DMA (SBUF partition-broadcast is not supported).
"""
import sys

sys.path.insert(0, '/opt/trn_rl_repo')

import numpy as np
import ml_dtypes

N_CORES = 8
N_TOTAL = 262144
ROWS = N_TOTAL // N_CORES        # 32768 rows per core
RBLK = 512                       # rows per block (PSUM bank width in fp32)
CF = 256
EMB = 256
NUM_SEASONS = 8192

_cache = {}


def _force_single_act_table(bacc):
    """Pin all activations to natural_log_exp_and_others (it contains every
    func this kernel uses: Prelu/Exp/Ln/Relu/Copy). The default chooser picks
    the first table per func, which alternates exp_and_others <-> natural_log
    and costs 2x 1.3us ACT_TABLE_LOAD per block."""
    from concourse.hw_specs import get_activation_tables as _real
    import functools

    @functools.cache
    def _only(arch):
        tabs = _real(arch)
        return {name: (s if name == "natural_log_exp_and_others" else set())
                for name, s in tabs.items()}

    bacc.get_activation_tables = _only


def _build_nc(rows, sim_safe=False):
    import concourse.bacc as bacc
    import concourse.tile as tile
    import concourse.mybir as mybir

    if not sim_safe:
        _force_single_act_table(bacc)

    dt = mybir.dt
    op = mybir.AluOpType
    act = mybir.ActivationFunctionType
    bf = dt.bfloat16
    nblk = rows // RBLK

    nc = bacc.Bacc("TRN2", target_bir_lowering=False, debug=False,
                   num_devices=N_CORES)

    xin_d = nc.dram_tensor("xin", [1024, rows], bf, kind="ExternalInput")
    wbig_d = nc.dram_tensor("wbig", [128, 1024], bf, kind="ExternalInput")
    we_d = nc.dram_tensor("we", [128, 18], bf, kind="ExternalInput")
    wout_d = nc.dram_tensor("woutt", [128, 2], bf, kind="ExternalInput")
    bbig_d = nc.dram_tensor("bbig", [128, 2], dt.float32, kind="ExternalInput")
    ones3_d = nc.dram_tensor("ones3", [3, 1], bf, kind="ExternalInput")
    sel12_d = nc.dram_tensor("sel12", [3, 2], bf, kind="ExternalInput")
    neg1_d = nc.dram_tensor("neg1", [1, 2], bf, kind="ExternalInput")
    yout_d = nc.dram_tensor("yout", [nblk, RBLK], dt.float32,
                            kind="ExternalOutput")

    with tile.TileContext(nc) as tc:
        with (
            tc.tile_pool(name="consts", bufs=1) as cpool,
            tc.tile_pool(name="xin", bufs=6) as xpool,
            tc.tile_pool(name="work", bufs=2) as wpool,
            tc.tile_pool(name="dscr", bufs=2, space="DRAM") as dpool,
            tc.tile_pool(name="ps_e", bufs=1, space="PSUM") as ps_e,
            tc.tile_pool(name="ps_z1", bufs=1, space="PSUM") as ps_z1,
            tc.tile_pool(name="ps_el2", bufs=1, space="PSUM") as ps_el2,
            tc.tile_pool(name="ps_z", bufs=2, space="PSUM") as ps_z,
            tc.tile_pool(name="ps_y", bufs=1, space="PSUM") as ps_y,
        ):
            wbig_s = cpool.tile([128, 4, 256], bf)
            nc.sync.dma_start(wbig_s[:], wbig_d[:].rearrange("p (k m) -> p k m", k=4))
            we_s = cpool.tile([128, 6, 3], bf)
            nc.sync.dma_start(we_s[:], we_d[:].rearrange("p (j c) -> p j c", j=6))
            wout_s = cpool.tile([128, 2], bf)
            nc.sync.dma_start(wout_s[:], wout_d[:])
            bbig_s = cpool.tile([128, 2], dt.float32)
            nc.sync.dma_start(bbig_s[:], bbig_d[:])
            ones3_s = cpool.tile([3, 1], bf)
            nc.sync.dma_start(ones3_s[:], ones3_d[:])
            sel12_s = cpool.tile([3, 2], bf)
            nc.sync.dma_start(sel12_s[:], sel12_d[:])
            neg1_s = cpool.tile([1, 2], bf)
            nc.sync.dma_start(neg1_s[:], neg1_d[:])

            xin_ap = xin_d[:].rearrange("(k p) n -> p k n", p=128)

            def stage_a(i):
                """Alpha stage for block i: load + attention logits + softmax
                weights, broadcast across partitions. Issued one block ahead so
                the PE has independent work queued while block i-1's main
                matmuls wait on agg."""
                c0 = i * RBLK
                # chunk order: 0,1=I  2,3=H  4,5=D1  6,7=D2
                xblk = xpool.tile([128, 8, RBLK], bf, tag="xblk")
                nc.sync.dma_start(xblk[:], xin_ap[:, :, c0:c0 + RBLK])

                # e.T [3, R]: attention logits (lhsT cols: e1, e2, e3)
                e_ps = ps_e.tile([3, RBLK], dt.float32, tag="e")
                for idx, (j, k) in enumerate([(0, 2), (1, 3), (2, 4),
                                              (3, 5), (4, 6), (5, 7)]):
                    nc.tensor.matmul(e_ps[:], we_s[:, j, :], xblk[:, k, :],
                                     start=(idx == 0), stop=(idx == 5))

                # el = leaky_relu(e)
                el = wpool.tile([3, RBLK], bf, tag="el")
                if sim_safe:
                    ec = wpool.tile([3, RBLK], dt.float32, tag="ec")
                    nc.vector.tensor_scalar_mul(ec[:], e_ps[:], 0.01)
                    nc.vector.tensor_tensor(el[:], e_ps[:], ec[:], op=op.max)
                else:
                    nc.scalar.activation(el[:], e_ps[:], act.Prelu, alpha=0.01)

                # t = exp(el); Z = sum_c t; alpha_c = exp(el_c - lnZ)
                t_s = wpool.tile([3, RBLK], bf, tag="t")
                nc.scalar.activation(t_s[:], el[:], act.Exp)
                z_ps = ps_z1.tile([1, RBLK], dt.float32, tag="z1")
                nc.tensor.matmul(z_ps[:], ones3_s[:], t_s[:],
                                 start=True, stop=True)
                lnz = wpool.tile([1, RBLK], bf, tag="lnz")
                nc.scalar.activation(lnz[:], z_ps[:], act.Ln)
                el2_ps = ps_el2.tile([2, RBLK], dt.float32, tag="el2")
                nc.tensor.matmul(el2_ps[:], sel12_s[:], el[:],
                                 start=True, stop=False)
                nc.tensor.matmul(el2_ps[:], neg1_s[:], lnz[:],
                                 start=False, stop=True)
                al = wpool.tile([2, RBLK], bf, tag="al")
                nc.scalar.activation(al[:], el2_ps[:], act.Exp)

                # broadcast alphas across partitions via DRAM bounce
                scr = dpool.tile([2, RBLK], bf, tag="scr")
                nc.scalar.dma_start(scr[:], al[:])
                a12 = wpool.tile([128, 2, RBLK], bf, tag="a12")
                nc.scalar.dma_start(
                    a12[:], scr[:].unsqueeze(0).broadcast_to([128, 2, RBLK]))
                return xblk, a12

            def stage_b(i, xblk, a12):
                """Main stage for block i: agg combine, big matmul, relu, y."""
                # agg.T = H.T + a1*D1.T + a2*D2.T  (both chunks at once)
                a1v = a12[:, 0:1, :].broadcast_to([128, 2, RBLK])
                a2v = a12[:, 1:2, :].broadcast_to([128, 2, RBLK])
                m1 = wpool.tile([128, 2, RBLK], bf, tag="m1")
                nc.vector.tensor_tensor(m1[:], xblk[:, 4:6, :], a1v, op=op.mult)
                m2 = wpool.tile([128, 2, RBLK], bf, tag="m2")
                nc.vector.tensor_tensor(m2[:], xblk[:, 6:8, :], a2v, op=op.mult)
                s1 = wpool.tile([128, 2, RBLK], bf, tag="s1")
                nc.vector.tensor_tensor(s1[:], m1[:], m2[:], op=op.add)
                agg = wpool.tile([128, 2, RBLK], bf, tag="agg")
                nc.gpsimd.tensor_tensor(agg[:], s1[:], xblk[:, 2:4, :],
                                        op=op.add)

                # z.T = Wbig @ [I; agg].T; m = relu(z + bbig); y = wout . m
                z_ps = ps_z.tile([128, 2, RBLK], dt.float32, tag="z")
                for m in range(2):
                    for k in range(4):
                        rhs = xblk[:, k, :] if k < 2 else agg[:, k - 2, :]
                        nc.tensor.matmul(z_ps[:, m, :],
                                         wbig_s[:, k, m * 128:(m + 1) * 128],
                                         rhs, start=(k == 0), stop=(k == 3))
                mres = wpool.tile([128, 2, RBLK], bf, tag="mres")
                for m in range(2):
                    nc.scalar.activation(mres[:, m, :], z_ps[:, m, :], act.Relu,
                                         bias=bbig_s[:, m:m + 1])

                y_ps = ps_y.tile([1, RBLK], dt.float32, tag="y")
                for m in range(2):
                    nc.tensor.matmul(y_ps[:], wout_s[:, m:m + 1], mres[:, m, :],
                                     start=(m == 0), stop=(m == 1))
                ys = wpool.tile([1, RBLK], dt.float32, tag="ys")
                nc.vector.tensor_copy(ys[:], y_ps[:])
                nc.scalar.dma_start(yout_d[i:i + 1, :], ys[:])

            pend = None
            for i in range(nblk):
                cur = stage_a(i)
                if pend is not None:
                    stage_b(i - 1, *pend)
                pend = cur
            stage_b(nblk - 1, *pend)

    nc.compile()
    return nc


def _get_nc(rows):
    if rows not in _cache:
        _cache[rows] = _build_nc(rows)
    return _cache[rows]


def _host_prep(indiv_f, hierarchy_f, strength_f, recency_f,
               w_attn1, Wv, bv, Wo, bo, Wfc, bfc, Wout):
    """Fold weights and build the device-side arrays (full, unsharded)."""
    f32 = np.float32
    bf16 = ml_dtypes.bfloat16
    indiv_f = np.asarray(indiv_f, f32)
    H = np.asarray(hierarchy_f, f32)
    S = np.asarray(strength_f, f32)
    R = np.asarray(recency_f, f32)
    n = indiv_f.shape[0]

    w1 = np.asarray(w_attn1, f32).reshape(-1)
    w1H, w1S = w1[:CF], w1[CF:]
    u = w1H + w1S

    Wv64 = np.asarray(Wv, np.float64)
    Wo64 = np.asarray(Wo, np.float64)
    Wfc64 = np.asarray(Wfc, np.float64)
    Wbig = (Wfc64 @ Wo64 @ Wv64).astype(f32)                    # [256, 512]
    bbig = (Wfc64 @ (Wo64 @ np.asarray(bv, np.float64)
                     + np.asarray(bo, np.float64))
            + np.asarray(bfc, np.float64)).astype(f32)          # [256]
    wout = np.asarray(Wout, f32).reshape(-1)                    # [256]

    X = np.empty((1024, n), bf16)
    X[0:256] = indiv_f.T
    X[256:512] = H.T
    X[512:768] = (S - H).T
    X[768:1024] = (R - H).T

    wbig_t = np.ascontiguousarray(
        Wbig.T.reshape(4, 128, 256).transpose(1, 0, 2).reshape(128, 1024)
    ).astype(bf16)
    we = np.zeros((128, 6, 3), f32)
    we[:, 0, :] = u[0:128, None]
    we[:, 1, :] = u[128:256, None]
    we[:, 2, 0] = w1S[0:128]
    we[:, 3, 0] = w1S[128:256]
    we[:, 4, 1] = w1S[0:128]
    we[:, 5, 1] = w1S[128:256]
    we = we.reshape(128, 18).astype(bf16)
    wout_t = np.ascontiguousarray(wout.reshape(2, 128).T).astype(bf16)
    bbig2 = np.ascontiguousarray(bbig.reshape(2, 128).T)
    ones3 = np.ones((3, 1), bf16)
    sel12 = np.zeros((3, 2), np.float32)
    sel12[0, 0] = 1.0
    sel12[1, 1] = 1.0
    neg1 = -np.ones((1, 2), np.float32)
    return X, dict(wbig=wbig_t, we=we, woutt=wout_t, bbig=bbig2,
                   ones3=ones3, sel12=sel12.astype(bf16),
                   neg1=neg1.astype(bf16))


_last_result = None


def kernel(indiv_f, hierarchy_f, strength_f, recency_f, season_ids,
           w_attn1, Wq, bq, Wk, bk, Wv, bv, Wo, bo, Wfc, bfc, Wout, bout,
           **_unused):
    from concourse.bass_utils import run_bass_kernel_spmd

    X, consts = _host_prep(indiv_f, hierarchy_f, strength_f, recency_f,
                           w_attn1, Wv, bv, Wo, bo, Wfc, bfc, Wout)

    nc = _get_nc(ROWS)
    in_maps = []
    for c in range(N_CORES):
        m = dict(consts)
        m["xin"] = X[:, c * ROWS:(c + 1) * ROWS]
        in_maps.append(m)
    res = run_bass_kernel_spmd(nc, in_maps, core_ids=list(range(N_CORES)))
    global _last_result
    _last_result = res
    y = np.concatenate([res.results[c]["yout"].reshape(-1)
                        for c in range(N_CORES)])

    ids = np.asarray(season_ids).reshape(-1)
    sums = np.bincount(ids, weights=y.astype(np.float64),
                       minlength=NUM_SEASONS)
    counts = np.bincount(ids, minlength=NUM_SEASONS)
    out = sums / np.maximum(counts, 1) + float(np.asarray(bout).reshape(-1)[0])
    return out.astype(np.float32).reshape(NUM_SEASONS, 1)


# revision 12
# speedup vs baseline: 1.8871x; 1.1415x over previous
"""Trainium2 Bass kernel for nn_Nonhier_Optmatch (8-core SPMD, bf16).

Contract: kernel(**inputs) takes the FULL unsharded inputs and returns the
FULL [8192, 1] float32 output. Internally shards the N=262144 row axis
across 8 NeuronCores.

Math notes (exact reformulations of the reference):
  - The MHA softmax is over a size-1 axis => attn weight == 1.0, so q/k/Wq/Wk
    are dead code and attn_out = (x @ Wv.T + bv) @ Wo.T + bo.
  - Linear folding: member_emb = relu(x @ Wbig.T + bbig) with
    Wbig = Wfc @ Wo @ Wv and bbig = Wfc @ (Wo @ bv + bo) + bfc.
  - Output fold: out[s] = (sum_{i in s} relu(z_i) . wout) / count_s + bout,
    so only the scalar y_i = relu(z_i) . wout leaves the device; the ragged
    per-season mean over scalars is a host bincount.
  - With D1 = S - H, D2 = R - H and u = w1H + w1S:
      e1 = u.H + w1S.D1, e2 = u.H + w1S.D2, e3 = u.H
      agg = H + a1*D1 + a2*D2   (a = softmax(leaky_relu(e)))
  - Division-free softmax: a_c = exp(prelu(e_c) - ln(sum_c exp(prelu(e_c))));
    the -lnZ subtraction rides a PSUM-accumulating matmul, so ACT only needs
    {Prelu, Exp, Ln, Relu, Copy} - all in one HW activation table.

Device layout: feature-major (activations transposed on host) so the feature
contractions run on the PE. Per-row alphas are broadcast across partitions by
a DRAM-bounce DMA (SBUF partition-broadcast is not supported).
"""
import sys

sys.path.insert(0, '/opt/trn_rl_repo')

import numpy as np
import ml_dtypes

N_CORES = 8
N_TOTAL = 262144
ROWS = N_TOTAL // N_CORES        # 32768 rows per core
RBLK = 512                       # rows per block (PSUM bank width in fp32)
CF = 256
EMB = 256
NUM_SEASONS = 8192

_cache = {}


def _force_single_act_table(bacc):
    """Pin all activations to natural_log_exp_and_others (it contains every
    func this kernel uses: Prelu/Exp/Ln/Relu/Copy). The default chooser picks
    the first table per func, which alternates exp_and_others <-> natural_log
    and costs 2x 1.3us ACT_TABLE_LOAD per block."""
    from concourse.hw_specs import get_activation_tables as _real
    import functools

    @functools.cache
    def _only(arch):
        tabs = _real(arch)
        return {name: (s if name == "natural_log_exp_and_others" else set())
                for name, s in tabs.items()}

    bacc.get_activation_tables = _only


def _build_nc(rows, sim_safe=False):
    import concourse.bacc as bacc
    import concourse.tile as tile
    import concourse.mybir as mybir

    if not sim_safe:
        _force_single_act_table(bacc)

    dt = mybir.dt
    op = mybir.AluOpType
    act = mybir.ActivationFunctionType
    bf = dt.bfloat16
    nblk = rows // RBLK

    nc = bacc.Bacc("TRN2", target_bir_lowering=False, debug=False,
                   num_devices=N_CORES)

    xin_d = nc.dram_tensor("xin", [1024, rows], bf, kind="ExternalInput")
    wbig_d = nc.dram_tensor("wbig", [128, 1024], bf, kind="ExternalInput")
    we_d = nc.dram_tensor("we", [128, 18], bf, kind="ExternalInput")
    wout_d = nc.dram_tensor("woutt", [128, 2], bf, kind="ExternalInput")
    bbig_d = nc.dram_tensor("bbig", [128, 2], dt.float32, kind="ExternalInput")
    ones3_d = nc.dram_tensor("ones3", [3, 1], bf, kind="ExternalInput")
    sel12_d = nc.dram_tensor("sel12", [3, 2], bf, kind="ExternalInput")
    neg1_d = nc.dram_tensor("neg1", [1, 2], bf, kind="ExternalInput")
    yout_d = nc.dram_tensor("yout", [nblk, RBLK], dt.float32,
                            kind="ExternalOutput")

    with tile.TileContext(nc) as tc:
        with (
            tc.tile_pool(name="consts", bufs=1) as cpool,
            tc.tile_pool(name="xin", bufs=6) as xpool,
            tc.tile_pool(name="work", bufs=2) as wpool,
            tc.tile_pool(name="worka", bufs=3) as apool,
            tc.tile_pool(name="dscr", bufs=3, space="DRAM") as dpool,
            tc.tile_pool(name="ps_e", bufs=1, space="PSUM") as ps_e,
            tc.tile_pool(name="ps_z1", bufs=1, space="PSUM") as ps_z1,
            tc.tile_pool(name="ps_el2", bufs=1, space="PSUM") as ps_el2,
            tc.tile_pool(name="ps_z", bufs=2, space="PSUM") as ps_z,
            tc.tile_pool(name="ps_y", bufs=1, space="PSUM") as ps_y,
        ):
            wbig_s = cpool.tile([128, 4, 256], bf)
            nc.sync.dma_start(wbig_s[:], wbig_d[:].rearrange("p (k m) -> p k m", k=4))
            we_s = cpool.tile([128, 6, 3], bf)
            nc.sync.dma_start(we_s[:], we_d[:].rearrange("p (j c) -> p j c", j=6))
            wout_s = cpool.tile([128, 2], bf)
            nc.sync.dma_start(wout_s[:], wout_d[:])
            bbig_s = cpool.tile([128, 2], dt.float32)
            nc.sync.dma_start(bbig_s[:], bbig_d[:])
            ones3_s = cpool.tile([3, 1], bf)
            nc.sync.dma_start(ones3_s[:], ones3_d[:])
            sel12_s = cpool.tile([3, 2], bf)
            nc.sync.dma_start(sel12_s[:], sel12_d[:])
            neg1_s = cpool.tile([1, 2], bf)
            nc.sync.dma_start(neg1_s[:], neg1_d[:])

            xin_ap = xin_d[:].rearrange("(k p) n -> p k n", p=128)

            def stage_a(i):
                """Alpha stage for block i: load + attention logits + softmax
                weights, broadcast across partitions. Issued one block ahead so
                the PE has independent work queued while block i-1's main
                matmuls wait on agg."""
                c0 = i * RBLK
                # chunk order: 0,1=I  2,3=H  4,5=D1  6,7=D2
                xblk = xpool.tile([128, 8, RBLK], bf, tag="xblk")
                nc.sync.dma_start(xblk[:], xin_ap[:, :, c0:c0 + RBLK])

                # e.T [3, R]: attention logits (lhsT cols: e1, e2, e3)
                e_ps = ps_e.tile([3, RBLK], dt.float32, tag="e")
                for idx, (j, k) in enumerate([(0, 2), (1, 3), (2, 4),
                                              (3, 5), (4, 6), (5, 7)]):
                    nc.tensor.matmul(e_ps[:], we_s[:, j, :], xblk[:, k, :],
                                     start=(idx == 0), stop=(idx == 5))

                # el = leaky_relu(e)
                el = apool.tile([3, RBLK], bf, tag="el")
                if sim_safe:
                    ec = apool.tile([3, RBLK], dt.float32, tag="ec")
                    nc.vector.tensor_scalar_mul(ec[:], e_ps[:], 0.01)
                    nc.vector.tensor_tensor(el[:], e_ps[:], ec[:], op=op.max)
                else:
                    nc.scalar.activation(el[:], e_ps[:], act.Prelu, alpha=0.01)

                # t = exp(el); Z = sum_c t; alpha_c = exp(el_c - lnZ)
                t_s = apool.tile([3, RBLK], bf, tag="t")
                nc.scalar.activation(t_s[:], el[:], act.Exp)
                z_ps = ps_z1.tile([1, RBLK], dt.float32, tag="z1")
                nc.tensor.matmul(z_ps[:], ones3_s[:], t_s[:],
                                 start=True, stop=True)
                lnz = apool.tile([1, RBLK], bf, tag="lnz")
                nc.scalar.activation(lnz[:], z_ps[:], act.Ln)
                el2_ps = ps_el2.tile([2, RBLK], dt.float32, tag="el2")
                nc.tensor.matmul(el2_ps[:], sel12_s[:], el[:],
                                 start=True, stop=False)
                nc.tensor.matmul(el2_ps[:], neg1_s[:], lnz[:],
                                 start=False, stop=True)
                al = apool.tile([2, RBLK], bf, tag="al")
                nc.scalar.activation(al[:], el2_ps[:], act.Exp)

                # broadcast alphas across partitions via DRAM bounce
                scr = dpool.tile([2, RBLK], bf, tag="scr")
                nc.gpsimd.dma_start(scr[:], al[:])
                a12 = apool.tile([128, 2, RBLK], bf, tag="a12")
                nc.gpsimd.dma_start(
                    a12[:], scr[:].unsqueeze(0).broadcast_to([128, 2, RBLK]))
                return xblk, a12

            def stage_b(i, xblk, a12):
                """Main stage for block i: agg combine, big matmul, relu, y."""
                # agg.T = H.T + a1*D1.T + a2*D2.T  (both chunks at once)
                a1v = a12[:, 0:1, :].broadcast_to([128, 2, RBLK])
                a2v = a12[:, 1:2, :].broadcast_to([128, 2, RBLK])
                m1 = wpool.tile([128, 2, RBLK], bf, tag="m1")
                nc.vector.tensor_tensor(m1[:], xblk[:, 4:6, :], a1v, op=op.mult)
                m2 = wpool.tile([128, 2, RBLK], bf, tag="m2")
                nc.vector.tensor_tensor(m2[:], xblk[:, 6:8, :], a2v, op=op.mult)
                s1 = wpool.tile([128, 2, RBLK], bf, tag="s1")
                nc.vector.tensor_tensor(s1[:], m1[:], m2[:], op=op.add)
                agg = wpool.tile([128, 2, RBLK], bf, tag="agg")
                nc.vector.tensor_tensor(agg[:], s1[:], xblk[:, 2:4, :],
                                        op=op.add)

                # z.T = Wbig @ [I; agg].T; m = relu(z + bbig); y = wout . m
                z_ps = ps_z.tile([128, 2, RBLK], dt.float32, tag="z")
                for m in range(2):
                    for k in range(4):
                        rhs = xblk[:, k, :] if k < 2 else agg[:, k - 2, :]
                        nc.tensor.matmul(z_ps[:, m, :],
                                         wbig_s[:, k, m * 128:(m + 1) * 128],
                                         rhs, start=(k == 0), stop=(k == 3))
                mres = wpool.tile([128, 2, RBLK], bf, tag="mres")
                for m in range(2):
                    nc.scalar.activation(mres[:, m, :], z_ps[:, m, :], act.Relu,
                                         bias=bbig_s[:, m:m + 1])

                y_ps = ps_y.tile([1, RBLK], dt.float32, tag="y")
                for m in range(2):
                    nc.tensor.matmul(y_ps[:], wout_s[:, m:m + 1], mres[:, m, :],
                                     start=(m == 0), stop=(m == 1))
                ys = wpool.tile([1, RBLK], dt.float32, tag="ys")
                nc.vector.tensor_copy(ys[:], y_ps[:])
                nc.sync.dma_start(yout_d[i:i + 1, :], ys[:])

            LOOKAHEAD = 2
            pend = {}
            for i in range(nblk):
                pend[i] = stage_a(i)
                if i >= LOOKAHEAD:
                    stage_b(i - LOOKAHEAD, *pend.pop(i - LOOKAHEAD))
            for i in range(nblk - LOOKAHEAD, nblk):
                stage_b(i, *pend.pop(i))

    nc.compile()
    return nc


def _get_nc(rows):
    if rows not in _cache:
        _cache[rows] = _build_nc(rows)
    return _cache[rows]


def _host_prep(indiv_f, hierarchy_f, strength_f, recency_f,
               w_attn1, Wv, bv, Wo, bo, Wfc, bfc, Wout):
    """Fold weights and build the device-side arrays (full, unsharded)."""
    f32 = np.float32
    bf16 = ml_dtypes.bfloat16
    indiv_f = np.asarray(indiv_f, f32)
    H = np.asarray(hierarchy_f, f32)
    S = np.asarray(strength_f, f32)
    R = np.asarray(recency_f, f32)
    n = indiv_f.shape[0]

    w1 = np.asarray(w_attn1, f32).reshape(-1)
    w1H, w1S = w1[:CF], w1[CF:]
    u = w1H + w1S

    Wv64 = np.asarray(Wv, np.float64)
    Wo64 = np.asarray(Wo, np.float64)
    Wfc64 = np.asarray(Wfc, np.float64)
    Wbig = (Wfc64 @ Wo64 @ Wv64).astype(f32)                    # [256, 512]
    bbig = (Wfc64 @ (Wo64 @ np.asarray(bv, np.float64)
                     + np.asarray(bo, np.float64))
            + np.asarray(bfc, np.float64)).astype(f32)          # [256]
    wout = np.asarray(Wout, f32).reshape(-1)                    # [256]

    X = np.empty((1024, n), bf16)
    X[0:256] = indiv_f.T
    X[256:512] = H.T
    X[512:768] = (S - H).T
    X[768:1024] = (R - H).T

    wbig_t = np.ascontiguousarray(
        Wbig.T.reshape(4, 128, 256).transpose(1, 0, 2).reshape(128, 1024)
    ).astype(bf16)
    we = np.zeros((128, 6, 3), f32)
    we[:, 0, :] = u[0:128, None]
    we[:, 1, :] = u[128:256, None]
    we[:, 2, 0] = w1S[0:128]
    we[:, 3, 0] = w1S[128:256]
    we[:, 4, 1] = w1S[0:128]
    we[:, 5, 1] = w1S[128:256]
    we = we.reshape(128, 18).astype(bf16)
    wout_t = np.ascontiguousarray(wout.reshape(2, 128).T).astype(bf16)
    bbig2 = np.ascontiguousarray(bbig.reshape(2, 128).T)
    ones3 = np.ones((3, 1), bf16)
    sel12 = np.zeros((3, 2), np.float32)
    sel12[0, 0] = 1.0
    sel12[1, 1] = 1.0
    neg1 = -np.ones((1, 2), np.float32)
    return X, dict(wbig=wbig_t, we=we, woutt=wout_t, bbig=bbig2,
                   ones3=ones3, sel12=sel12.astype(bf16),
                   neg1=neg1.astype(bf16))


_last_result = None


def kernel(indiv_f, hierarchy_f, strength_f, recency_f, season_ids,
           w_attn1, Wq, bq, Wk, bk, Wv, bv, Wo, bo, Wfc, bfc, Wout, bout,
           **_unused):
    from concourse.bass_utils import run_bass_kernel_spmd

    X, consts = _host_prep(indiv_f, hierarchy_f, strength_f, recency_f,
                           w_attn1, Wv, bv, Wo, bo, Wfc, bfc, Wout)

    nc = _get_nc(ROWS)
    in_maps = []
    for c in range(N_CORES):
        m = dict(consts)
        m["xin"] = X[:, c * ROWS:(c + 1) * ROWS]
        in_maps.append(m)
    res = run_bass_kernel_spmd(nc, in_maps, core_ids=list(range(N_CORES)))
    global _last_result
    _last_result = res
    y = np.concatenate([res.results[c]["yout"].reshape(-1)
                        for c in range(N_CORES)])

    ids = np.asarray(season_ids).reshape(-1)
    sums = np.bincount(ids, weights=y.astype(np.float64),
                       minlength=NUM_SEASONS)
    counts = np.bincount(ids, minlength=NUM_SEASONS)
    out = sums / np.maximum(counts, 1) + float(np.asarray(bout).reshape(-1)[0])
    return out.astype(np.float32).reshape(NUM_SEASONS, 1)


# revision 13
# speedup vs baseline: 2.1904x; 1.1607x over previous
"""Trainium2 Bass kernel for nn_Nonhier_Optmatch (8-core SPMD, bf16).

Contract: kernel(**inputs) takes the FULL unsharded inputs and returns the
FULL [8192, 1] float32 output. Internally shards the N=262144 row axis
across 8 NeuronCores.

Math notes (exact reformulations of the reference):
  - The MHA softmax is over a size-1 axis => attn weight == 1.0, so q/k/Wq/Wk
    are dead code and attn_out = (x @ Wv.T + bv) @ Wo.T + bo.
  - Linear folding: member_emb = relu(x @ Wbig.T + bbig) with
    Wbig = Wfc @ Wo @ Wv and bbig = Wfc @ (Wo @ bv + bo) + bfc.
  - Output fold: out[s] = (sum_{i in s} relu(z_i) . wout) / count_s + bout,
    so only the scalar y_i = relu(z_i) . wout leaves the device; the ragged
    per-season mean over scalars is a host bincount.
  - With D1 = S - H, D2 = R - H and u = w1H + w1S:
      e1 = u.H + w1S.D1, e2 = u.H + w1S.D2, e3 = u.H
      agg = H + a1*D1 + a2*D2   (a = softmax(leaky_relu(e)))
  - Division-free softmax: a_c = exp(prelu(e_c) - ln(sum_c exp(prelu(e_c))));
    the -lnZ subtraction rides a PSUM-accumulating matmul, so ACT only needs
    {Prelu, Exp, Ln, Relu, Copy} - all in one HW activation table.

Device layout: feature-major (activations transposed on host) so the feature
contractions run on the PE. Per-row alphas are broadcast across partitions by
a DRAM-bounce DMA (SBUF partition-broadcast is not supported).
"""
import sys

sys.path.insert(0, '/opt/trn_rl_repo')

import numpy as np
import ml_dtypes

N_CORES = 8
N_TOTAL = 262144
ROWS = N_TOTAL // N_CORES        # 32768 rows per core
RBLK = 512                       # rows per block (PSUM bank width in fp32)
CF = 256
EMB = 256
NUM_SEASONS = 8192

_cache = {}


def _force_single_act_table(bacc):
    """Pin all activations to natural_log_exp_and_others (it contains every
    func this kernel uses: Prelu/Exp/Ln/Relu/Copy). The default chooser picks
    the first table per func, which alternates exp_and_others <-> natural_log
    and costs 2x 1.3us ACT_TABLE_LOAD per block."""
    from concourse.hw_specs import get_activation_tables as _real
    import functools

    @functools.cache
    def _only(arch):
        tabs = _real(arch)
        return {name: (s if name == "natural_log_exp_and_others" else set())
                for name, s in tabs.items()}

    bacc.get_activation_tables = _only


def _build_nc(rows, sim_safe=False):
    import concourse.bacc as bacc
    import concourse.tile as tile
    import concourse.mybir as mybir

    if not sim_safe:
        _force_single_act_table(bacc)

    dt = mybir.dt
    op = mybir.AluOpType
    act = mybir.ActivationFunctionType
    bf = dt.bfloat16
    nblk = rows // RBLK

    nc = bacc.Bacc("TRN2", target_bir_lowering=False, debug=False,
                   num_devices=N_CORES)

    xin_d = nc.dram_tensor("xin", [1024, rows], bf, kind="ExternalInput")
    wbig_d = nc.dram_tensor("wbig", [128, 1024], bf, kind="ExternalInput")
    we_d = nc.dram_tensor("we", [128, 18], bf, kind="ExternalInput")
    wout_d = nc.dram_tensor("woutt", [128, 2], bf, kind="ExternalInput")
    bbig_d = nc.dram_tensor("bbig", [128, 2], dt.float32, kind="ExternalInput")
    ones3_d = nc.dram_tensor("ones3", [3, 1], bf, kind="ExternalInput")
    sel12_d = nc.dram_tensor("sel12", [3, 2], bf, kind="ExternalInput")
    neg1_d = nc.dram_tensor("neg1", [1, 2], bf, kind="ExternalInput")
    yout_d = nc.dram_tensor("yout", [nblk, RBLK], dt.float32,
                            kind="ExternalOutput")

    with tile.TileContext(nc) as tc:
        with (
            tc.tile_pool(name="consts", bufs=1) as cpool,
            tc.tile_pool(name="xin", bufs=8) as xpool,
            tc.tile_pool(name="work", bufs=2) as wpool,
            tc.tile_pool(name="worka", bufs=4) as apool,
            tc.tile_pool(name="dscr", bufs=4, space="DRAM") as dpool,
            tc.tile_pool(name="ps_e", bufs=1, space="PSUM") as ps_e,
            tc.tile_pool(name="ps_z1", bufs=1, space="PSUM") as ps_z1,
            tc.tile_pool(name="ps_el2", bufs=1, space="PSUM") as ps_el2,
            tc.tile_pool(name="ps_z", bufs=2, space="PSUM") as ps_z,
            tc.tile_pool(name="ps_y", bufs=1, space="PSUM") as ps_y,
        ):
            wbig_s = cpool.tile([128, 4, 256], bf)
            nc.sync.dma_start(wbig_s[:], wbig_d[:].rearrange("p (k m) -> p k m", k=4))
            we_s = cpool.tile([128, 6, 3], bf)
            nc.sync.dma_start(we_s[:], we_d[:].rearrange("p (j c) -> p j c", j=6))
            wout_s = cpool.tile([128, 2], bf)
            nc.sync.dma_start(wout_s[:], wout_d[:])
            bbig_s = cpool.tile([128, 2], dt.float32)
            nc.sync.dma_start(bbig_s[:], bbig_d[:])
            ones3_s = cpool.tile([3, 1], bf)
            nc.sync.dma_start(ones3_s[:], ones3_d[:])
            sel12_s = cpool.tile([3, 2], bf)
            nc.sync.dma_start(sel12_s[:], sel12_d[:])
            neg1_s = cpool.tile([1, 2], bf)
            nc.sync.dma_start(neg1_s[:], neg1_d[:])

            xin_ap = xin_d[:].rearrange("(k p) n -> p k n", p=128)

            def stage_a(i):
                """Alpha stage for block i: load + attention logits + softmax
                weights, broadcast across partitions. Issued one block ahead so
                the PE has independent work queued while block i-1's main
                matmuls wait on agg."""
                c0 = i * RBLK
                # chunk order: 0,1=I  2,3=H  4,5=D1  6,7=D2
                xblk = xpool.tile([128, 8, RBLK], bf, tag="xblk")
                nc.sync.dma_start(xblk[:], xin_ap[:, :, c0:c0 + RBLK])

                # e.T [3, R]: attention logits (lhsT cols: e1, e2, e3)
                e_ps = ps_e.tile([3, RBLK], dt.float32, tag="e")
                for idx, (j, k) in enumerate([(0, 2), (1, 3), (2, 4),
                                              (3, 5), (4, 6), (5, 7)]):
                    nc.tensor.matmul(e_ps[:], we_s[:, j, :], xblk[:, k, :],
                                     start=(idx == 0), stop=(idx == 5))

                # el = leaky_relu(e)
                el = apool.tile([3, RBLK], bf, tag="el")
                if sim_safe:
                    ec = apool.tile([3, RBLK], dt.float32, tag="ec")
                    nc.vector.tensor_scalar_mul(ec[:], e_ps[:], 0.01)
                    nc.vector.tensor_tensor(el[:], e_ps[:], ec[:], op=op.max)
                else:
                    nc.scalar.activation(el[:], e_ps[:], act.Prelu, alpha=0.01)

                # t = exp(el); Z = sum_c t; alpha_c = exp(el_c - lnZ)
                t_s = apool.tile([3, RBLK], bf, tag="t")
                nc.scalar.activation(t_s[:], el[:], act.Exp)
                z_ps = ps_z1.tile([1, RBLK], dt.float32, tag="z1")
                nc.tensor.matmul(z_ps[:], ones3_s[:], t_s[:],
                                 start=True, stop=True)
                lnz = apool.tile([1, RBLK], bf, tag="lnz")
                nc.scalar.activation(lnz[:], z_ps[:], act.Ln)
                el2_ps = ps_el2.tile([2, RBLK], dt.float32, tag="el2")
                nc.tensor.matmul(el2_ps[:], sel12_s[:], el[:],
                                 start=True, stop=False)
                nc.tensor.matmul(el2_ps[:], neg1_s[:], lnz[:],
                                 start=False, stop=True)
                al = apool.tile([2, RBLK], bf, tag="al")
                nc.scalar.activation(al[:], el2_ps[:], act.Exp)

                # broadcast alphas across partitions via DRAM bounce
                scr = dpool.tile([2, RBLK], bf, tag="scr")
                nc.gpsimd.dma_start(scr[:], al[:])
                a12 = apool.tile([128, 2, RBLK], bf, tag="a12")
                nc.gpsimd.dma_start(
                    a12[:], scr[:].unsqueeze(0).broadcast_to([128, 2, RBLK]))
                return xblk, a12

            def stage_b(i, xblk, a12):
                """Main stage for block i: agg combine, big matmul, relu, y."""
                # agg.T = H.T + a1*D1.T + a2*D2.T  (both chunks at once)
                a1v = a12[:, 0:1, :].broadcast_to([128, 2, RBLK])
                a2v = a12[:, 1:2, :].broadcast_to([128, 2, RBLK])
                m1 = wpool.tile([128, 2, RBLK], bf, tag="m1")
                nc.vector.tensor_tensor(m1[:], xblk[:, 4:6, :], a1v, op=op.mult)
                m2 = wpool.tile([128, 2, RBLK], bf, tag="m2")
                nc.vector.tensor_tensor(m2[:], xblk[:, 6:8, :], a2v, op=op.mult)
                s1 = wpool.tile([128, 2, RBLK], bf, tag="s1")
                nc.vector.tensor_tensor(s1[:], m1[:], m2[:], op=op.add)
                agg = wpool.tile([128, 2, RBLK], bf, tag="agg")
                nc.vector.tensor_tensor(agg[:], s1[:], xblk[:, 2:4, :],
                                        op=op.add)

                # z.T = Wbig @ [I; agg].T; m = relu(z + bbig); y = wout . m
                z_ps = ps_z.tile([128, 2, RBLK], dt.float32, tag="z")
                for m in range(2):
                    for k in range(4):
                        rhs = xblk[:, k, :] if k < 2 else agg[:, k - 2, :]
                        nc.tensor.matmul(z_ps[:, m, :],
                                         wbig_s[:, k, m * 128:(m + 1) * 128],
                                         rhs, start=(k == 0), stop=(k == 3))
                mres = wpool.tile([128, 2, RBLK], bf, tag="mres")
                for m in range(2):
                    nc.scalar.activation(mres[:, m, :], z_ps[:, m, :], act.Relu,
                                         bias=bbig_s[:, m:m + 1])

                y_ps = ps_y.tile([1, RBLK], dt.float32, tag="y")
                for m in range(2):
                    nc.tensor.matmul(y_ps[:], wout_s[:, m:m + 1], mres[:, m, :],
                                     start=(m == 0), stop=(m == 1))
                ys = wpool.tile([1, RBLK], dt.float32, tag="ys")
                nc.vector.tensor_copy(ys[:], y_ps[:])
                nc.sync.dma_start(yout_d[i:i + 1, :], ys[:])

            LOOKAHEAD = 3
            pend = {}
            for i in range(nblk):
                pend[i] = stage_a(i)
                if i >= LOOKAHEAD:
                    stage_b(i - LOOKAHEAD, *pend.pop(i - LOOKAHEAD))
            for i in range(nblk - LOOKAHEAD, nblk):
                stage_b(i, *pend.pop(i))

    nc.compile()
    return nc


def _get_nc(rows):
    if rows not in _cache:
        _cache[rows] = _build_nc(rows)
    return _cache[rows]


def _host_prep(indiv_f, hierarchy_f, strength_f, recency_f,
               w_attn1, Wv, bv, Wo, bo, Wfc, bfc, Wout):
    """Fold weights and build the device-side arrays (full, unsharded)."""
    f32 = np.float32
    bf16 = ml_dtypes.bfloat16
    indiv_f = np.asarray(indiv_f, f32)
    H = np.asarray(hierarchy_f, f32)
    S = np.asarray(strength_f, f32)
    R = np.asarray(recency_f, f32)
    n = indiv_f.shape[0]

    w1 = np.asarray(w_attn1, f32).reshape(-1)
    w1H, w1S = w1[:CF], w1[CF:]
    u = w1H + w1S

    Wv64 = np.asarray(Wv, np.float64)
    Wo64 = np.asarray(Wo, np.float64)
    Wfc64 = np.asarray(Wfc, np.float64)
    Wbig = (Wfc64 @ Wo64 @ Wv64).astype(f32)                    # [256, 512]
    bbig = (Wfc64 @ (Wo64 @ np.asarray(bv, np.float64)
                     + np.asarray(bo, np.float64))
            + np.asarray(bfc, np.float64)).astype(f32)          # [256]
    wout = np.asarray(Wout, f32).reshape(-1)                    # [256]

    X = np.empty((1024, n), bf16)
    X[0:256] = indiv_f.T
    X[256:512] = H.T
    X[512:768] = (S - H).T
    X[768:1024] = (R - H).T

    wbig_t = np.ascontiguousarray(
        Wbig.T.reshape(4, 128, 256).transpose(1, 0, 2).reshape(128, 1024)
    ).astype(bf16)
    we = np.zeros((128, 6, 3), f32)
    we[:, 0, :] = u[0:128, None]
    we[:, 1, :] = u[128:256, None]
    we[:, 2, 0] = w1S[0:128]
    we[:, 3, 0] = w1S[128:256]
    we[:, 4, 1] = w1S[0:128]
    we[:, 5, 1] = w1S[128:256]
    we = we.reshape(128, 18).astype(bf16)
    wout_t = np.ascontiguousarray(wout.reshape(2, 128).T).astype(bf16)
    bbig2 = np.ascontiguousarray(bbig.reshape(2, 128).T)
    ones3 = np.ones((3, 1), bf16)
    sel12 = np.zeros((3, 2), np.float32)
    sel12[0, 0] = 1.0
    sel12[1, 1] = 1.0
    neg1 = -np.ones((1, 2), np.float32)
    return X, dict(wbig=wbig_t, we=we, woutt=wout_t, bbig=bbig2,
                   ones3=ones3, sel12=sel12.astype(bf16),
                   neg1=neg1.astype(bf16))


_last_result = None


def kernel(indiv_f, hierarchy_f, strength_f, recency_f, season_ids,
           w_attn1, Wq, bq, Wk, bk, Wv, bv, Wo, bo, Wfc, bfc, Wout, bout,
           **_unused):
    from concourse.bass_utils import run_bass_kernel_spmd

    X, consts = _host_prep(indiv_f, hierarchy_f, strength_f, recency_f,
                           w_attn1, Wv, bv, Wo, bo, Wfc, bfc, Wout)

    nc = _get_nc(ROWS)
    in_maps = []
    for c in range(N_CORES):
        m = dict(consts)
        m["xin"] = X[:, c * ROWS:(c + 1) * ROWS]
        in_maps.append(m)
    res = run_bass_kernel_spmd(nc, in_maps, core_ids=list(range(N_CORES)))
    global _last_result
    _last_result = res
    y = np.concatenate([res.results[c]["yout"].reshape(-1)
                        for c in range(N_CORES)])

    ids = np.asarray(season_ids).reshape(-1)
    sums = np.bincount(ids, weights=y.astype(np.float64),
                       minlength=NUM_SEASONS)
    counts = np.bincount(ids, minlength=NUM_SEASONS)
    out = sums / np.maximum(counts, 1) + float(np.asarray(bout).reshape(-1)[0])
    return out.astype(np.float32).reshape(NUM_SEASONS, 1)


# revision 15
# speedup vs baseline: 2.3140x; 1.0564x over previous
"""Trainium2 Bass kernel for nn_Nonhier_Optmatch (8-core SPMD, bf16).

Contract: kernel(**inputs) takes the FULL unsharded inputs and returns the
FULL [8192, 1] float32 output. Internally shards the N=262144 row axis
across 8 NeuronCores.

Math notes (exact reformulations of the reference):
  - The MHA softmax is over a size-1 axis => attn weight == 1.0, so q/k/Wq/Wk
    are dead code and attn_out = (x @ Wv.T + bv) @ Wo.T + bo.
  - Linear folding: member_emb = relu(x @ Wbig.T + bbig) with
    Wbig = Wfc @ Wo @ Wv and bbig = Wfc @ (Wo @ bv + bo) + bfc.
  - Output fold: out[s] = (sum_{i in s} relu(z_i) . wout) / count_s + bout,
    so only the scalar y_i = relu(z_i) . wout leaves the device; the ragged
    per-season mean over scalars is a host bincount.
  - With D1 = S - H, D2 = R - H and u = w1H + w1S:
      e1 = u.H + w1S.D1, e2 = u.H + w1S.D2, e3 = u.H
      agg = H + a1*D1 + a2*D2   (a = softmax(leaky_relu(e)))
  - Division-free softmax: a_c = exp(prelu(e_c) - ln(sum_c exp(prelu(e_c))));
    the -lnZ subtraction rides a PSUM-accumulating matmul, so ACT only needs
    {Prelu, Exp, Ln, Relu, Copy} - all in one HW activation table.

Device layout: feature-major (activations transposed on host) so the feature
contractions run on the PE. Per-row alphas are broadcast across partitions by
a DRAM-bounce DMA (SBUF partition-broadcast is not supported).
"""
import sys

sys.path.insert(0, '/opt/trn_rl_repo')

import numpy as np
import ml_dtypes

N_CORES = 8
N_TOTAL = 262144
ROWS = N_TOTAL // N_CORES        # 32768 rows per core
RBLK = 512                       # rows per block (PSUM bank width in fp32)
CF = 256
EMB = 256
NUM_SEASONS = 8192

_cache = {}


def _force_single_act_table(bacc):
    """Pin all activations to natural_log_exp_and_others (it contains every
    func this kernel uses: Prelu/Exp/Ln/Relu/Copy). The default chooser picks
    the first table per func, which alternates exp_and_others <-> natural_log
    and costs 2x 1.3us ACT_TABLE_LOAD per block."""
    from concourse.hw_specs import get_activation_tables as _real
    import functools

    @functools.cache
    def _only(arch):
        tabs = _real(arch)
        return {name: (s if name == "natural_log_exp_and_others" else set())
                for name, s in tabs.items()}

    bacc.get_activation_tables = _only


def _enable_ldw_opt():
    """The framework compiles with --enable-ldw-opt=false, which leaves every
    LDWEIGHTS serialized with its MATMUL (~90ns per matmul). Rewrite the flag
    on the walrus command line."""
    import concourse.bass_utils as bu
    if getattr(bu, "_ldw_opt_patched", False):
        return
    orig = bu.run_command

    def patched(argv, **kw):
        argv = ["--enable-ldw-opt=true" if a == "--enable-ldw-opt=false" else a
                for a in argv]
        return orig(argv, **kw)

    bu.run_command = patched
    bu._ldw_opt_patched = True


def _build_nc(rows, sim_safe=False):
    import concourse.bacc as bacc
    import concourse.tile as tile
    import concourse.mybir as mybir

    if not sim_safe:
        _force_single_act_table(bacc)

    dt = mybir.dt
    op = mybir.AluOpType
    act = mybir.ActivationFunctionType
    bf = dt.bfloat16
    nblk = rows // RBLK

    nc = bacc.Bacc("TRN2", target_bir_lowering=False, debug=False,
                   num_devices=N_CORES)

    xin_d = nc.dram_tensor("xin", [1024, rows], bf, kind="ExternalInput")
    wbig_d = nc.dram_tensor("wbig", [128, 1024], bf, kind="ExternalInput")
    we_d = nc.dram_tensor("we", [128, 18], bf, kind="ExternalInput")
    wout_d = nc.dram_tensor("woutt", [128, 2], bf, kind="ExternalInput")
    bbig_d = nc.dram_tensor("bbig", [128, 2], dt.float32, kind="ExternalInput")
    ones3_d = nc.dram_tensor("ones3", [3, 1], bf, kind="ExternalInput")
    sel12_d = nc.dram_tensor("sel12", [3, 2], bf, kind="ExternalInput")
    neg1_d = nc.dram_tensor("neg1", [1, 2], bf, kind="ExternalInput")
    yout_d = nc.dram_tensor("yout", [nblk, RBLK], dt.float32,
                            kind="ExternalOutput")

    with tile.TileContext(nc) as tc:
        with (
            tc.tile_pool(name="consts", bufs=1) as cpool,
            tc.tile_pool(name="xin", bufs=8) as xpool,
            tc.tile_pool(name="work", bufs=2) as wpool,
            tc.tile_pool(name="worka", bufs=5) as apool,
            tc.tile_pool(name="dscr", bufs=5, space="DRAM") as dpool,
            tc.tile_pool(name="ps_e", bufs=1, space="PSUM") as ps_e,
            tc.tile_pool(name="ps_z1", bufs=1, space="PSUM") as ps_z1,
            tc.tile_pool(name="ps_el2", bufs=1, space="PSUM") as ps_el2,
            tc.tile_pool(name="ps_z", bufs=2, space="PSUM") as ps_z,
            tc.tile_pool(name="ps_y", bufs=1, space="PSUM") as ps_y,
        ):
            wbig_s = cpool.tile([128, 4, 256], bf)
            nc.sync.dma_start(wbig_s[:], wbig_d[:].rearrange("p (k m) -> p k m", k=4))
            we_s = cpool.tile([128, 6, 3], bf)
            nc.sync.dma_start(we_s[:], we_d[:].rearrange("p (j c) -> p j c", j=6))
            wout_s = cpool.tile([128, 2], bf)
            nc.sync.dma_start(wout_s[:], wout_d[:])
            bbig_s = cpool.tile([128, 2], dt.float32)
            nc.sync.dma_start(bbig_s[:], bbig_d[:])
            ones3_s = cpool.tile([3, 1], bf)
            nc.sync.dma_start(ones3_s[:], ones3_d[:])
            sel12_s = cpool.tile([3, 2], bf)
            nc.sync.dma_start(sel12_s[:], sel12_d[:])
            neg1_s = cpool.tile([1, 2], bf)
            nc.sync.dma_start(neg1_s[:], neg1_d[:])

            xin_ap = xin_d[:].rearrange("(k p) n -> p k n", p=128)

            def stage_a(i):
                """Alpha stage for block i: load + attention logits + softmax
                weights, broadcast across partitions. Issued one block ahead so
                the PE has independent work queued while block i-1's main
                matmuls wait on agg."""
                c0 = i * RBLK
                # chunk order: 0,1=I  2,3=H  4,5=D1  6,7=D2
                xblk = xpool.tile([128, 8, RBLK], bf, tag="xblk")
                nc.sync.dma_start(xblk[:], xin_ap[:, :, c0:c0 + RBLK])

                # e.T [3, R]: attention logits (lhsT cols: e1, e2, e3)
                e_ps = ps_e.tile([3, RBLK], dt.float32, tag="e")
                for idx, (j, k) in enumerate([(0, 2), (1, 3), (2, 4),
                                              (3, 5), (4, 6), (5, 7)]):
                    nc.tensor.matmul(e_ps[:], we_s[:, j, :], xblk[:, k, :],
                                     start=(idx == 0), stop=(idx == 5))

                # el = leaky_relu(e)
                el = apool.tile([3, RBLK], bf, tag="el")
                if sim_safe:
                    ec = apool.tile([3, RBLK], dt.float32, tag="ec")
                    nc.vector.tensor_scalar_mul(ec[:], e_ps[:], 0.01)
                    nc.vector.tensor_tensor(el[:], e_ps[:], ec[:], op=op.max)
                else:
                    nc.scalar.activation(el[:], e_ps[:], act.Prelu, alpha=0.01)

                # t = exp(el); Z = sum_c t; alpha_c = exp(el_c - lnZ)
                t_s = apool.tile([3, RBLK], bf, tag="t")
                nc.scalar.activation(t_s[:], el[:], act.Exp)
                z_ps = ps_z1.tile([1, RBLK], dt.float32, tag="z1")
                nc.tensor.matmul(z_ps[:], ones3_s[:], t_s[:],
                                 start=True, stop=True)
                lnz = apool.tile([1, RBLK], bf, tag="lnz")
                nc.scalar.activation(lnz[:], z_ps[:], act.Ln)
                el2_ps = ps_el2.tile([2, RBLK], dt.float32, tag="el2")
                nc.tensor.matmul(el2_ps[:], sel12_s[:], el[:],
                                 start=True, stop=False)
                nc.tensor.matmul(el2_ps[:], neg1_s[:], lnz[:],
                                 start=False, stop=True)
                al = apool.tile([2, RBLK], bf, tag="al")
                nc.scalar.activation(al[:], el2_ps[:], act.Exp)

                # broadcast alphas across partitions via DRAM bounce
                scr = dpool.tile([2, RBLK], bf, tag="scr")
                nc.gpsimd.dma_start(scr[:], al[:])
                a12 = apool.tile([128, 2, RBLK], bf, tag="a12")
                nc.gpsimd.dma_start(
                    a12[:], scr[:].unsqueeze(0).broadcast_to([128, 2, RBLK]))
                return xblk, a12

            def stage_b(i, xblk, a12):
                """Main stage for block i: agg combine, big matmul, relu, y."""
                # agg.T = H.T + a1*D1.T + a2*D2.T  (both chunks at once)
                a1v = a12[:, 0:1, :].broadcast_to([128, 2, RBLK])
                a2v = a12[:, 1:2, :].broadcast_to([128, 2, RBLK])
                m1 = wpool.tile([128, 2, RBLK], bf, tag="m1")
                nc.vector.tensor_tensor(m1[:], xblk[:, 4:6, :], a1v, op=op.mult)
                m2 = wpool.tile([128, 2, RBLK], bf, tag="m2")
                nc.vector.tensor_tensor(m2[:], xblk[:, 6:8, :], a2v, op=op.mult)
                s1 = wpool.tile([128, 2, RBLK], bf, tag="s1")
                nc.vector.tensor_tensor(s1[:], m1[:], m2[:], op=op.add)
                agg = wpool.tile([128, 2, RBLK], bf, tag="agg")
                nc.vector.tensor_tensor(agg[:], s1[:], xblk[:, 2:4, :],
                                        op=op.add)

                # z.T = Wbig @ [I; agg].T; m = relu(z + bbig); y = wout . m
                z_ps = ps_z.tile([128, 2, RBLK], dt.float32, tag="z")
                for m in range(2):
                    for k in range(4):
                        rhs = xblk[:, k, :] if k < 2 else agg[:, k - 2, :]
                        nc.tensor.matmul(z_ps[:, m, :],
                                         wbig_s[:, k, m * 128:(m + 1) * 128],
                                         rhs, start=(k == 0), stop=(k == 3))
                mres = wpool.tile([128, 2, RBLK], bf, tag="mres")
                for m in range(2):
                    nc.scalar.activation(mres[:, m, :], z_ps[:, m, :], act.Relu,
                                         bias=bbig_s[:, m:m + 1])

                y_ps = ps_y.tile([1, RBLK], dt.float32, tag="y")
                for m in range(2):
                    nc.tensor.matmul(y_ps[:], wout_s[:, m:m + 1], mres[:, m, :],
                                     start=(m == 0), stop=(m == 1))
                ys = wpool.tile([1, RBLK], dt.float32, tag="ys")
                nc.vector.tensor_copy(ys[:], y_ps[:])
                nc.sync.dma_start(yout_d[i:i + 1, :], ys[:])

            LOOKAHEAD = 4
            pend = {}
            for i in range(nblk):
                pend[i] = stage_a(i)
                if i >= LOOKAHEAD:
                    stage_b(i - LOOKAHEAD, *pend.pop(i - LOOKAHEAD))
            for i in range(nblk - LOOKAHEAD, nblk):
                stage_b(i, *pend.pop(i))

    nc.compile()
    return nc


def _get_nc(rows):
    if rows not in _cache:
        _cache[rows] = _build_nc(rows)
    return _cache[rows]


def _host_prep(indiv_f, hierarchy_f, strength_f, recency_f,
               w_attn1, Wv, bv, Wo, bo, Wfc, bfc, Wout):
    """Fold weights and build the device-side arrays (full, unsharded)."""
    f32 = np.float32
    bf16 = ml_dtypes.bfloat16
    indiv_f = np.asarray(indiv_f, f32)
    H = np.asarray(hierarchy_f, f32)
    S = np.asarray(strength_f, f32)
    R = np.asarray(recency_f, f32)
    n = indiv_f.shape[0]

    w1 = np.asarray(w_attn1, f32).reshape(-1)
    w1H, w1S = w1[:CF], w1[CF:]
    u = w1H + w1S

    Wv64 = np.asarray(Wv, np.float64)
    Wo64 = np.asarray(Wo, np.float64)
    Wfc64 = np.asarray(Wfc, np.float64)
    Wbig = (Wfc64 @ Wo64 @ Wv64).astype(f32)                    # [256, 512]
    bbig = (Wfc64 @ (Wo64 @ np.asarray(bv, np.float64)
                     + np.asarray(bo, np.float64))
            + np.asarray(bfc, np.float64)).astype(f32)          # [256]
    wout = np.asarray(Wout, f32).reshape(-1)                    # [256]

    X = np.empty((1024, n), bf16)
    X[0:256] = indiv_f.T
    X[256:512] = H.T
    X[512:768] = (S - H).T
    X[768:1024] = (R - H).T

    wbig_t = np.ascontiguousarray(
        Wbig.T.reshape(4, 128, 256).transpose(1, 0, 2).reshape(128, 1024)
    ).astype(bf16)
    we = np.zeros((128, 6, 3), f32)
    we[:, 0, :] = u[0:128, None]
    we[:, 1, :] = u[128:256, None]
    we[:, 2, 0] = w1S[0:128]
    we[:, 3, 0] = w1S[128:256]
    we[:, 4, 1] = w1S[0:128]
    we[:, 5, 1] = w1S[128:256]
    we = we.reshape(128, 18).astype(bf16)
    wout_t = np.ascontiguousarray(wout.reshape(2, 128).T).astype(bf16)
    bbig2 = np.ascontiguousarray(bbig.reshape(2, 128).T)
    ones3 = np.ones((3, 1), bf16)
    sel12 = np.zeros((3, 2), np.float32)
    sel12[0, 0] = 1.0
    sel12[1, 1] = 1.0
    neg1 = -np.ones((1, 2), np.float32)
    return X, dict(wbig=wbig_t, we=we, woutt=wout_t, bbig=bbig2,
                   ones3=ones3, sel12=sel12.astype(bf16),
                   neg1=neg1.astype(bf16))


_last_result = None


def kernel(indiv_f, hierarchy_f, strength_f, recency_f, season_ids,
           w_attn1, Wq, bq, Wk, bk, Wv, bv, Wo, bo, Wfc, bfc, Wout, bout,
           **_unused):
    from concourse.bass_utils import run_bass_kernel_spmd

    X, consts = _host_prep(indiv_f, hierarchy_f, strength_f, recency_f,
                           w_attn1, Wv, bv, Wo, bo, Wfc, bfc, Wout)

    nc = _get_nc(ROWS)
    in_maps = []
    for c in range(N_CORES):
        m = dict(consts)
        m["xin"] = X[:, c * ROWS:(c + 1) * ROWS]
        in_maps.append(m)
    res = run_bass_kernel_spmd(nc, in_maps, core_ids=list(range(N_CORES)))
    global _last_result
    _last_result = res
    y = np.concatenate([res.results[c]["yout"].reshape(-1)
                        for c in range(N_CORES)])

    ids = np.asarray(season_ids).reshape(-1)
    sums = np.bincount(ids, weights=y.astype(np.float64),
                       minlength=NUM_SEASONS)
    counts = np.bincount(ids, minlength=NUM_SEASONS)
    out = sums / np.maximum(counts, 1) + float(np.asarray(bout).reshape(-1)[0])
    return out.astype(np.float32).reshape(NUM_SEASONS, 1)
